# revision 1
# baseline (speedup 1.0000x reference)
"""Trainium2 Bass kernel for nn_Attention (B=4, N=2048, C=768, H=12, D=64).

Sharding: core c -> batch b=c//2, head-group hg=c%2 (6 heads each).
qkv_w column-parallel, proj_w row-parallel (host sums the 2 partials per b).

v4 structure (vs the f32r baseline):
  - QKV matmuls stay f32r (full rate, moving dims >= 256).
  - q/k tiles, rope tables, exp'd scores (es), V, attention output and the
    projection all run in bf16: transposes cost 1.0 cycles/row and the
    psum<->sbuf copies hit the DVE 2x half-word mode.
  - AV is FLIPPED: es [k,q] is the stationary operand, [v|1] the moving one,
    so each 128x128 score block costs 65 PE rows instead of 512. The ones
    column gives the softmax denominator in psum column 64. Output is
    token-major [q, d]; a bf16 PE transpose + copy rebuilds feature-major oT
    for the projection.
  - A matmul `start` zeroes its whole 2KB psum bank, so the 4 q-subtiles of a
    unit accumulate as qt0/qt1 in two ping-pong banks streaming with the
    exps, then qt2/qt3 re-use those banks; the qt0/1 drain, qt2/3
    accumulation and the transposes are deferred into the NEXT unit (staged
    after its first scores) so the PE never idles behind exp(kp7).
  - exp runs on ACT except 2 of 8 kpairs per unit, which run on DVE as ONE
    tensor_scalar: i16 = trunc(s*c0 + c1) bitcast to bf16 is a Schraudolph
    exp (max rel err ~3% on those tiles; end-to-end ~9e-3 vs 2e-2 budget).
  - GPSIMD cannot touch PSUM: Pool only carries sbuf-side rope math
    (sq/t_/qn); all psum exits live on DVE/ACT.
  - rsqrt via bit-trick seed + 1 Newton step (0.2% worst-case, q-side
    cancels in softmax).
  - No max-subtraction needed: RMSNorm bounds the logits (|z| <= ~16).
"""
import sys

sys.path.insert(0, "/opt/trn_rl_repo")

import numpy as np
import concourse.bass as bass
import concourse.mybir as mybir
import concourse.tile as tile
from concourse import bacc
from concourse.bass_utils import run_bass_kernel_spmd
from concourse.masks import make_identity

dt = mybir.dt
AF = mybir.ActivationFunctionType
ALU = mybir.AluOpType
AX = mybir.AxisListType

B, N, C = 4, 2048, 768
H, D = 12, 64
HPC = 6            # heads per core
EPS = 1e-6
NT = N // 128      # 16 token tiles
NCHUNK = C // 128  # 6 contraction chunks
SCALE = D ** -0.5  # 0.125
NG = 4             # qi groups
G = N // NG        # 512 per group
# bf16 Schraudolph exp: i16 = trunc(s*SCH_C0 + SCH_C1); bitcast bf16 ~ exp(s/8)
SCH_C0 = 184.664965 * SCALE
SCH_C1 = 16250.5
import os
# kpair indices whose exp runs on DVE via Schraudolph ("" = none)
SCH_KP = tuple(int(x) for x in os.environ.get("SCH_KP", "4,7").split(",") if x != "")
FILL_KP = tuple(int(x) for x in os.environ.get("FILL_KP", "0,2,5,6").split(","))
AV_TRAIL = int(os.environ.get("AV_TRAIL", "5"))
PROJ_KP = tuple(int(x) for x in os.environ.get("PROJ_KP", "4,5,6,7").split(","))
ONE_NR = int(os.environ.get("ONE_NR", "1"))
M2_POOL = int(os.environ.get("M2_POOL", "0"))
SCH_DEFER = int(os.environ.get("SCH_DEFER", "0"))
FLUSH_LAG = int(os.environ.get("FLUSH_LAG", "2"))
TAIL_KP = int(os.environ.get("TAIL_KP", "0"))
RAMP_N = int(os.environ.get("RAMP_N", "16"))
ES_BUFS = int(os.environ.get("ES_BUFS", "12"))


def _bc(ap, idx, count):
    """Insert a broadcast (step 0) free dim at position idx of an AP."""
    a = list(ap.ap)
    a.insert(idx, [0, count])
    return bass.AP(tensor=ap.tensor, offset=ap.offset, ap=a)


def build_program():
    nc = bacc.Bacc(None, target_bir_lowering=False)

    xT = nc.dram_tensor("xT", [C, N], dt.bfloat16, kind="ExternalInput")
    # host layout: [q0|k0 (256) | v (384) | q1|k1 | q2|k2]
    wqkvT = nc.dram_tensor("wqkvT", [C, 3 * HPC * D], dt.bfloat16, kind="ExternalInput")
    projT = nc.dram_tensor("projT", [HPC * D, C], dt.bfloat16, kind="ExternalInput")
    cqk = nc.dram_tensor("cqk", [N, 2 * D], dt.bfloat16, kind="ExternalInput")
    sqk = nc.dram_tensor("sqk", [N, 2 * D], dt.bfloat16, kind="ExternalInput")
    out = nc.dram_tensor("out", [N, C], dt.float32, kind="ExternalOutput")

    with tile.TileContext(nc) as tc:
        with (
            tc.tile_pool(name="persist", bufs=1) as persist,
            tc.tile_pool(name="qkrot", bufs=2) as qkrot,     # qT/kT rotate across pairs
            tc.tile_pool(name="work", bufs=3) as work,
            tc.tile_pool(name="qkblk", bufs=2) as qkblk,
            tc.tile_pool(name="tiny", bufs=2) as tiny,
            tc.tile_pool(name="den", bufs=2) as den,
            tc.tile_pool(name="p2e", bufs=ES_BUFS) as p2e,
            tc.tile_pool(name="outp", bufs=4) as outp,
            tc.tile_pool(name="psA", bufs=2, space="PSUM") as psA,   # qkv/tp/proj
            tc.tile_pool(name="psS", bufs=2, space="PSUM") as psS,   # scores
            tc.tile_pool(name="psV", bufs=2, space="PSUM") as psV,   # av + oT transposes
        ):
            # ---------------- persistent tiles --------------------------------
            oT = [[persist.tile([128, G], dt.bfloat16, name=f"oT{p}_{g}", tag=f"oT{p}_{g}")
                   for g in range(NG)] for p in range(3)]
            vA = [persist.tile([128, 4, HPC, D + 1], dt.bfloat16, name=f"vA{kg}", tag=f"vA{kg}")
                  for kg in range(NG)]
            identb = persist.tile([128, 128], dt.bfloat16, tag="identb")
            make_identity(nc, identb[:])
            ones1 = persist.tile([128, 1], dt.float32, tag="ones1")
            nc.vector.memset(ones1[:], 1.0)
            for kg in range(NG):
                nc.vector.tensor_copy(vA[kg][:, :, :, D : D + 1], _bc(_bc(ones1[:], 1, 4), 2, HPC))

            # weights / x^T / tables.  DMA order tuned so the first prep tiles
            # wait for the minimum byte set.
            xw_cm = tc.tile_pool(name="xw", bufs=1)
            xw = xw_cm.__enter__()
            wrA = []
            wrB = []
            for j in range(NCHUNK):
                wa = xw.tile([128, 640], dt.bfloat16, name=f"wrA{j}", tag=f"wrA{j}")
                nc.sync.dma_start(wa[:], wqkvT[j * 128 : (j + 1) * 128, 0:640])
                wrA.append(wa)
            xr = [[xw.tile([128, G], dt.bfloat16, name=f"xr{j}_{tg}", tag=f"xr{j}_{tg}")
                   for tg in range(NG)] for j in range(NCHUNK)]
            for j in range(NCHUNK):
                nc.sync.dma_start(xr[j][0][:], xT[j * 128 : (j + 1) * 128, 0:G])
            tabs = {}
            for name, dram in (("cqk", cqk), ("sqk", sqk)):
                t = persist.tile([128, NT, 2, D], dt.bfloat16, name=name, tag=name)
                nc.sync.dma_start(t[:], dram.rearrange("(t p) (qk d) -> p t qk d", p=128, qk=2))
                tabs[name] = t
            for tg in range(1, NG):
                for j in range(NCHUNK):
                    nc.sync.dma_start(xr[j][tg][:], xT[j * 128 : (j + 1) * 128, tg * G : (tg + 1) * G])
            for j in range(NCHUNK):
                wb = xw.tile([128, 512], dt.bfloat16, name=f"wrB{j}", tag=f"wrB{j}")
                nc.sync.dma_start(wb[:], wqkvT[j * 128 : (j + 1) * 128, 640:1152])
                wrB.append(wb)
            prW = []
            for p in range(3):
                wp = persist.tile([128, C], dt.bfloat16, name=f"prW{p}", tag=f"prW{p}")
                nc.sync.dma_start(wp[:], projT[p * 128 : (p + 1) * 128, :])
                prW.append(wp)

            # ------- interleaved emission: prep / attention / projection ------
            # Engines execute their instruction streams in order, so emission
            # order IS the schedule.

            def new_pair_state(p):
                # qkT[g]: columns 0:G hold q^T for qi-group g, G:2G hold k^T
                # for ki-group g.  bf16.
                return {
                    "p": p,
                    "qkT": [qkrot.tile([128, 2 * G], dt.bfloat16, name=f"qkT{p}_{g}", tag=f"qkT{g}") for g in range(NG)],
                    "pend": [],
                    "next": 0,
                }

            def flush_one(st):
                i, qn = st["pend"].pop(0)
                # both bf16 transposes land in one psum tile -> single 2x copy
                tp = psA.tile([128, 256], dt.bfloat16, tag="qkv")
                nc.tensor.transpose(tp[:, 0:128], qn[:, 0:128], identb[:])
                nc.tensor.transpose(tp[:, 128:256], qn[:, 128:256], identb[:])
                dst = st["qkT"][i // NG][:, :].rearrange("p (qk c) -> p qk c", qk=2)[
                    :, :, (i % NG) * 128 : (i % NG + 1) * 128]
                src_v = tp[:, :].rearrange("p (qk c) -> p qk c", qk=2)
                nc.vector.tensor_copy(dst, src_v)

            def prep_steps(st):
                # Split one prep tile into two PE bursts so the filler can
                # spread them between attention kpairs.
                p = st["p"]
                i = st["next"]
                st["next"] += 1
                hold = {}

                def s1():
                    if p == 0:
                        vp = psA.tile([128, HPC * D], dt.float32, tag="qkv")
                        for j in range(NCHUNK):
                            nc.tensor.matmul(vp[:], xr[j][i // NG][:, (i % NG) * 128 : (i % NG + 1) * 128],
                                             wrA[j][:, 256:640],
                                             start=(j == 0), stop=(j == NCHUNK - 1))
                        nc.scalar.copy(vA[i // NG][:, i % NG, :, 0:D], vp[:].rearrange("p (h d) -> p h d", h=HPC))
                    qkp = psA.tile([128, 256], dt.float32, tag="qkv")
                    for j in range(3):
                        wsl = wrA[j][:, 0:256] if p == 0 else wrB[j][:, (p - 1) * 256 : p * 256]
                        nc.tensor.matmul(qkp[:], xr[j][i // NG][:, (i % NG) * 128 : (i % NG + 1) * 128],
                                         wsl, start=(j == 0), stop=False)
                    hold["qkp"] = qkp

                def s2():
                    _finish_prep_tile(st, i, hold["qkp"])

                return [s1, s2]

            def _finish_prep_tile(st, i, qkp):
                p = st["p"]
                for j in range(3, NCHUNK):
                    wsl = wrA[j][:, 0:256] if p == 0 else wrB[j][:, (p - 1) * 256 : p * 256]
                    nc.tensor.matmul(qkp[:], xr[j][i // NG][:, (i % NG) * 128 : (i % NG + 1) * 128],
                                     wsl, start=False, stop=(j == NCHUNK - 1))
                if len(st["pend"]) >= FLUSH_LAG:
                    flush_one(st)
                qk_sb = qkblk.tile([128, 256], dt.bfloat16, tag="qk_sb")
                nc.scalar.copy(qk_sb[:], qkp[:])
                qk4 = qk_sb[:].rearrange("p (h d) -> p h d", h=4)
                # sum of squares per (token, slot).  During the pair-0 ramp
                # (no exps yet) the otherwise-idle ACT computes the squares
                # and DVE the rope add, halving Pool's prep throughput limit.
                ramp = (p == 0 and i < RAMP_N)
                sq = work.tile([128, 4, D], dt.bfloat16, tag="sq")
                if ramp:
                    nc.scalar.square(sq[:], qk4)
                else:
                    nc.gpsimd.tensor_tensor(sq[:], qk4, qk4, op=ALU.mult)
                ss = tiny.tile([128, 4], dt.float32, tag="ss16")
                nc.vector.tensor_reduce(ss[:], sq[:], axis=AX.X, op=ALU.add)
                # rsqrt on DVE (bit-trick + 2 Newton): nf = 1/sqrt(ss+D*EPS)
                ssh = tiny.tile([128, 4], dt.float32, tag="ssh")
                nc.vector.tensor_scalar(ssh[:], ss[:], 0.5, 0.5 * D * EPS,
                                        op0=ALU.mult, op1=ALU.add)
                y0i = tiny.tile([128, 4], dt.int32, tag="y0i")
                nc.vector.tensor_scalar(y0i[:], ss[:].bitcast(dt.int32), 1, 0,
                                        op0=ALU.logical_shift_right, op1=ALU.bitwise_or)
                nc.vector.tensor_scalar(y0i[:], y0i[:], -1, 0x5F3759DF,
                                        op0=ALU.mult, op1=ALU.add)
                nf16 = tiny.tile([128, 4], dt.float32, tag="nf16")
                yw = tiny.tile([128, 4], dt.float32, tag="yw")
                y = y0i[:].bitcast(dt.float32)
                for dst_ in (nf16,) if ONE_NR else (tiny.tile([128, 4], dt.float32, tag="y1"), nf16):
                    nc.vector.tensor_tensor(yw[:], y, y, op=ALU.mult)
                    nc.vector.tensor_tensor(yw[:], yw[:], ssh[:], op=ALU.mult)
                    nc.vector.tensor_scalar(yw[:], yw[:], -1.0, 1.5,
                                            op0=ALU.mult, op1=ALU.add)
                    nc.vector.tensor_tensor(dst_[:], y, yw[:], op=ALU.mult)
                    y = dst_[:]
                nfb = _bc(nf16[:], 2, D)
                t_ = work.tile([128, 4, D], dt.bfloat16, tag="t_")
                nc.gpsimd.tensor_tensor(t_[:], qk4, nfb, op=ALU.mult)
                # rope tables: [128, NT, 2(qk), D] bf16 with heads broadcast
                cwb = _bc(tabs["cqk"][:, i, :, :], 2, 2)
                swb = _bc(tabs["sqk"][:, i, :, :], 2, 2)
                t4 = t_[:].rearrange("p (qk h) d -> p qk h d", qk=2)
                h_ = D // 2
                m1 = work.tile([128, 2, 2, D], dt.bfloat16, tag="m1")
                nc.vector.tensor_tensor(m1[:], t4, cwb, op=ALU.mult)
                m2 = work.tile([128, 2, 2, D], dt.bfloat16, tag="m2")
                m2eng = nc.gpsimd if M2_POOL else nc.vector
                m2eng.tensor_tensor(m2[:, :, :, 0:h_], t4[:, :, :, h_:D], swb[:, :, :, 0:h_], op=ALU.mult)
                m2eng.tensor_tensor(m2[:, :, :, h_:D], t4[:, :, :, 0:h_], swb[:, :, :, h_:D], op=ALU.mult)
                qn = work.tile([128, 256], dt.bfloat16, tag="qn", bufs=4)
                qn_eng = nc.vector if ramp else nc.gpsimd
                qn_eng.tensor_tensor(qn[:].rearrange("p (qk h d) -> p qk h d", qk=2, h=2), m1[:], m2[:], op=ALU.add)
                st["pend"].append((i, qn))

            def emit_prep_tile(st):
                for s in prep_steps(st):
                    s()

            def finish_prep(st, step_q):
                while step_q:
                    step_q.pop(0)()
                while st["next"] < NT:
                    emit_prep_tile(st)
                while st["pend"]:
                    flush_one(st)

            def att_unit_gen(st, g, hh, oTps_box, tail_q):
                p = st["p"]
                h = 2 * p + hh
                off = 64 * hh
                # Two accumulation banks ping-pong across the 4 q-tiles: a
                # matmul start zeroes its whole 2KB psum bank, so concurrent
                # groups must sit in different banks.  qt0/qt1 stream with the
                # exps; qt2/qt3 re-use the banks at the unit end.
                av01 = [psV.tile([128, D + 1], dt.float32, name=f"av{qt}", tag="av",
                                 padded_shape=[128, 512]) for qt in range(2)]

                def emit_av(kpair, es, is_i16, qts, avt):
                    for half in range(2):
                        ki = kpair * 2 + half
                        for j, qt in enumerate(qts):
                            esb = es[:, half * 512 + qt * 128 : half * 512 + (qt + 1) * 128]
                            if is_i16:
                                esb = esb.bitcast(dt.bfloat16)
                            nc.tensor.matmul(
                                avt[j][:],
                                esb,
                                vA[ki // NG][:, ki % NG, h, :],
                                start=(ki == 0), stop=(ki == NT - 1),
                            )

                rd4 = den.tile([128, 4], dt.float32, tag="rd4")
                o_sb = den.tile([128, 4, D], dt.bfloat16, tag="o_sb")

                def normalize(qts, avt):
                    for j, qt in enumerate(qts):
                        nc.vector.reciprocal(rd4[:, qt : qt + 1], avt[j][:, D : D + 1])
                        nc.vector.tensor_scalar(o_sb[:, qt, :], avt[j][:, 0:D],
                                                rd4[:, qt : qt + 1], None, op0=ALU.mult)

                pend = []
                all_es = []
                for kpair in range(8):
                    sp = psS.tile([128, 1024], dt.float32, tag="sp")
                    for half in range(2):
                        ki = kpair * 2 + half
                        nc.tensor.matmul(
                            sp[:, half * 512 : (half + 1) * 512],
                            st["qkT"][ki // NG][off : off + 64, G + (ki % NG) * 128 : G + (ki % NG + 1) * 128],
                            st["qkT"][g][off : off + 64, 0:G],
                            start=True, stop=True,
                        )
                    # exp: mostly on ACT; SCH_KP kpairs on DVE via the bf16
                    # Schraudolph bit-trick (one tensor_scalar; GPSIMD cannot
                    # read PSUM so Pool is out).
                    if kpair in SCH_KP:
                        esi = p2e.tile([128, 1024], dt.int16, name="esi", tag="es")
                        nc.vector.tensor_scalar(esi[:], sp[:], SCH_C0, SCH_C1,
                                                op0=ALU.mult, op1=ALU.add)
                        entry = (kpair, esi, True)
                    else:
                        es = p2e.tile([128, 1024], dt.bfloat16, name="est", tag="es")
                        nc.scalar.activation(es[:], sp[:], AF.Exp, scale=SCALE)
                        entry = (kpair, es, False)
                    pend.append(entry)
                    all_es.append(entry)
                    if len(pend) > AV_TRAIL:
                        e = pend.pop(0)
                        emit_av(e[0], e[1], e[2], (0, 1), av01)
                    yield
                # The whole unit tail — qt0/qt1 drain, qt2/qt3 accumulation
                # and the feature-major transposes — is deferred into the NEXT
                # unit, staged after its first scores, so the PE never sits
                # behind exp(kp7) at a unit boundary.
                av23 = [psV.tile([128, D + 1], dt.float32, name=f"av{qt}", tag="av",
                                 padded_shape=[128, 512]) for qt in (2, 3)]

                def tail_av01():
                    for e in pend:
                        emit_av(e[0], e[1], e[2], (0, 1), av01)
                    normalize((0, 1), av01)

                def tail_av23():
                    for e in all_es:
                        emit_av(e[0], e[1], e[2], (2, 3), av23)
                    normalize((2, 3), av23)

                def tail_tp():
                    oTps = psA.tile([128, 4, 128], dt.bfloat16, name="oTps", tag="qkv")
                    for qt in range(4):
                        nc.tensor.transpose(oTps[off : off + 64, qt, :], o_sb[:, qt, :], identb[:])
                    nc.vector.tensor_copy(
                        oT[p][g][:, :].rearrange("p (qt c) -> p qt c", qt=4)[off : off + 64],
                        oTps[off : off + 64])
                tail_q.append((tail_av01, tail_av23, tail_tp))

            tail_q = []

            def drain_tails():
                while tail_q:
                    for f in tail_q.pop(0):
                        f()

            def drive_unit(st, g, hh, box, filler=None):
                # filler(kp) runs at every kpair boundary so prep/proj PE work
                # spreads inside the unit instead of bunching at its end; the
                # previous unit's deferred qt2/qt3 accumulation runs after this
                # unit's kp1 scores, its transposes one kpair later.
                stages = []
                for kp, _ in enumerate(att_unit_gen(st, g, hh, box, tail_q)):
                    if kp == 1:
                        while len(tail_q) > 1:
                            for f in tail_q.pop(0):
                                f()
                        if tail_q:
                            stages = list(tail_q.pop(0))
                    elif kp in (2, 3) and stages:
                        stages.pop(0)()
                    if kp == 1 and stages:
                        stages.pop(0)()
                    if filler is not None:
                        filler(kp)
                for f in stages:
                    f()

            def proj_steps(i):
                hold = {}

                def s1():
                    p512 = psA.tile([128, 512], dt.float32, tag="qkv")
                    for pp_ in range(3):
                        sl = oT[pp_][i // NG][:, (i % NG) * 128 : (i % NG + 1) * 128]
                        nc.tensor.matmul(p512[:], sl, prW[pp_][:, 0:512],
                                         start=(pp_ == 0), stop=(pp_ == 2))
                    os_ = outp.tile([128, C], dt.float32, tag="os")
                    nc.vector.tensor_copy(os_[:, 0:512], p512[:])
                    hold["os"] = os_

                def s2():
                    p256 = psA.tile([128, 256], dt.float32, tag="qkv")
                    for pp_ in range(3):
                        sl = oT[pp_][i // NG][:, (i % NG) * 128 : (i % NG + 1) * 128]
                        nc.tensor.matmul(p256[:], sl, prW[pp_][:, 512:768],
                                         start=(pp_ == 0), stop=(pp_ == 2))
                    os_ = hold["os"]
                    nc.vector.tensor_copy(os_[:, 512:768], p256[:])
                    nc.sync.dma_start(out[i * 128 : (i + 1) * 128, :], os_[:])

                return [s1, s2]

            def emit_proj_tile(i):
                for s in proj_steps(i):
                    s()

            # pair-0 prep up front (V matmuls included); the first attention
            # unit's kpairs are interleaved as soon as their kT/vA quads are
            # flushed, so the ACT exp stream starts early.
            cur = new_pair_state(0)
            box0 = {}
            gen0 = att_unit_gen(cur, 0, 0, box0, tail_q)
            gate = [max(3, 2 * j + 1) + FLUSH_LAG for j in range(8)]
            gate = [g if g <= NT - 1 else 99 for g in gate]
            kp_done = 0
            for i in range(NT):
                emit_prep_tile(cur)
                while kp_done < 8 and i >= gate[kp_done]:
                    next(gen0)
                    kp_done += 1
            finish_prep(cur, [])
            for _ in gen0:
                pass

            proj_queue = list(range(NT))
            for p in range(3):
                nxt = new_pair_state(p + 1) if p < 2 else None
                step_q = []

                def filler(g_cur, _nxt=nxt, _sq=step_q):
                    def f(kp):
                        if kp not in (FILL_KP if _nxt is not None else PROJ_KP):
                            return
                        if _nxt is not None:
                            if not _sq and _nxt["next"] < NT:
                                _sq.extend(prep_steps(_nxt))
                            if _sq:
                                _sq.pop(0)()
                        else:
                            if not _sq and proj_queue and proj_queue[0] < g_cur * NG:
                                _sq.extend(proj_steps(proj_queue.pop(0)))
                            if _sq:
                                _sq.pop(0)()
                    return f

                for g in range(NG):
                    box = box0 if (p == 0 and g == 0) else {}
                    for hh in range(2):
                        if p == 0 and g == 0 and hh == 0:
                            continue
                        drive_unit(cur, g, hh, box, filler(g))
                if nxt is not None:
                    finish_prep(nxt, step_q)
                    cur = nxt
                else:
                    while step_q:
                        step_q.pop(0)()
            drain_tails()
            last_steps = [proj_steps(i) for i in proj_queue]
            for s_idx in range(2):
                for ss in last_steps:
                    ss[s_idx]()
            xw_cm.__exit__(None, None, None)

    nc.compile()
    return nc


_NC = None


def _get_nc():
    global _NC
    if _NC is None:
        _NC = build_program()
    return _NC


def _prep_inputs(x, cos, sin, qkv_w, q_norm_w, k_norm_w, proj_w):
    import ml_dtypes
    bf16 = ml_dtypes.bfloat16
    cos2 = np.asarray(cos, np.float32).reshape(N, D // 2)
    sin2 = np.asarray(sin, np.float32).reshape(N, D // 2)
    cos_full = np.concatenate([cos2, cos2], axis=1)          # [N, 64]
    sin_signed = np.concatenate([-sin2, sin2], axis=1)       # [N, 64]

    def tables(w):
        w = np.asarray(w, np.float32)
        wswap = np.concatenate([w[D // 2 :], w[: D // 2]])
        cw = (8.0 * cos_full * w[None, :]).astype(np.float32)
        sw = (8.0 * sin_signed * wswap[None, :]).astype(np.float32)
        return np.ascontiguousarray(cw), np.ascontiguousarray(sw)

    cwq_, swq_ = tables(q_norm_w)
    cwk_, swk_ = tables(k_norm_w)
    cqk_ = np.ascontiguousarray(np.stack([cwq_, cwk_], axis=1).reshape(N, 2 * D)).astype(bf16)
    sqk_ = np.ascontiguousarray(np.stack([swq_, swk_], axis=1).reshape(N, 2 * D)).astype(bf16)

    in_maps = []
    for c in range(8):
        b, hg = c // 2, c % 2
        h0 = HPC * hg
        rows = np.r_[h0 * D : (h0 + HPC) * D]
        wq = qkv_w[rows]          # [384, C]
        wk = qkv_w[C + rows]
        wv = qkv_w[2 * C + rows]
        # pack as [q0|k0 (256), v (384), q1|k1, q2|k2]
        parts = [wq[0:128], wk[0:128], wv]
        for p in range(1, 3):
            parts.append(wq[p * 128 : (p + 1) * 128])
            parts.append(wk[p * 128 : (p + 1) * 128])
        wqkvT_ = np.ascontiguousarray(np.concatenate(parts, 0).T).astype(bf16)
        projT_ = np.ascontiguousarray(proj_w[:, rows].T).astype(bf16)
        xT_ = np.ascontiguousarray(x[b].T).astype(bf16)
        in_maps.append({
            "xT": xT_, "wqkvT": wqkvT_, "projT": projT_,
            "cqk": cqk_, "sqk": sqk_,
        })
    return in_maps


def kernel(x, cos, sin, qkv_w, q_norm_w, k_norm_w, proj_w, proj_b, _want_trace=False):
    x = np.asarray(x, np.float32)
    qkv_w = np.asarray(qkv_w, np.float32)
    proj_w = np.asarray(proj_w, np.float32)
    proj_b = np.asarray(proj_b, np.float32)
    in_maps = _prep_inputs(x, cos, sin, qkv_w, q_norm_w, k_norm_w, proj_w)
    nc = _get_nc()
    res = run_bass_kernel_spmd(nc, in_maps, core_ids=list(range(8)), trace=_want_trace)
    out = np.empty((B, N, C), np.float32)
    for b in range(B):
        out[b] = res.results[2 * b]["out"] + res.results[2 * b + 1]["out"] + proj_b[None, :]
    if _want_trace:
        return out, res
    return out



# revision 29
# speedup vs baseline: 1.0147x; 1.0147x over previous
"""Trainium2 Bass kernel for nn_Attention (B=4, N=2048, C=768, H=12, D=64).

Sharding: core c -> batch b=c//2, head-group hg=c%2 (6 heads each).
qkv_w column-parallel, proj_w row-parallel (host sums the 2 partials per b).

v4 structure (vs the f32r baseline):
  - QKV matmuls stay f32r (full rate, moving dims >= 256).
  - q/k tiles, rope tables, exp'd scores (es), V, attention output and the
    projection all run in bf16: transposes cost 1.0 cycles/row and the
    psum<->sbuf copies hit the DVE 2x half-word mode.
  - AV is FLIPPED: es [k,q] is the stationary operand, [v|1] the moving one,
    so each 128x128 score block costs 65 PE rows instead of 512. The ones
    column gives the softmax denominator in psum column 64. Output is
    token-major [q, d]; a bf16 PE transpose + copy rebuilds feature-major oT
    for the projection.
  - A matmul `start` zeroes its whole 2KB psum bank, so the 4 q-subtiles of a
    unit accumulate as qt0/qt1 in two ping-pong banks streaming with the
    exps, then qt2/qt3 re-use those banks; the qt0/1 drain, qt2/3
    accumulation and the transposes are deferred into the NEXT unit (staged
    after its first scores) so the PE never idles behind exp(kp7).
  - exp runs on ACT except 2 of 8 kpairs per unit, which run on DVE as ONE
    tensor_scalar: i16 = trunc(s*c0 + c1) bitcast to bf16 is a Schraudolph
    exp (max rel err ~3% on those tiles; end-to-end ~9e-3 vs 2e-2 budget).
  - GPSIMD cannot touch PSUM: Pool only carries sbuf-side rope math
    (sq/t_/qn); all psum exits live on DVE/ACT.
  - rsqrt via bit-trick seed + 1 Newton step (0.2% worst-case, q-side
    cancels in softmax).
  - No max-subtraction needed: RMSNorm bounds the logits (|z| <= ~16).
"""
import sys

sys.path.insert(0, "/opt/trn_rl_repo")

import numpy as np
import concourse.bass as bass
import concourse.mybir as mybir
import concourse.tile as tile
from concourse import bacc
from concourse.bass_utils import run_bass_kernel_spmd
from concourse.masks import make_identity

dt = mybir.dt
AF = mybir.ActivationFunctionType
ALU = mybir.AluOpType
AX = mybir.AxisListType

B, N, C = 4, 2048, 768
H, D = 12, 64
HPC = 6            # heads per core
EPS = 1e-6
NT = N // 128      # 16 token tiles
NCHUNK = C // 128  # 6 contraction chunks
SCALE = D ** -0.5  # 0.125
NG = 4             # qi groups
G = N // NG        # 512 per group
# bf16 Schraudolph exp: i16 = trunc(s*SCH_C0 + SCH_C1); bitcast bf16 ~ exp(s/8)
SCH_C0 = 184.664965 * SCALE
SCH_C1 = 16250.5
import os
# kpair indices whose exp runs on DVE via Schraudolph ("" = none)
SCH_KP = tuple(int(x) for x in os.environ.get("SCH_KP", "5,7").split(",") if x != "")
FILL_KP = tuple(int(x) for x in os.environ.get("FILL_KP", "0,2,5,6").split(","))
AV_TRAIL = int(os.environ.get("AV_TRAIL", "5"))
PROJ_KP = tuple(int(x) for x in os.environ.get("PROJ_KP", "4,5,6,7").split(","))
ONE_NR = int(os.environ.get("ONE_NR", "1"))
M2_POOL = int(os.environ.get("M2_POOL", "0"))
SCH_DEFER = int(os.environ.get("SCH_DEFER", "0"))
FLUSH_LAG = int(os.environ.get("FLUSH_LAG", "4"))
TAIL_KP = int(os.environ.get("TAIL_KP", "0"))
RAMP_N = int(os.environ.get("RAMP_N", "16"))
ES_BUFS = int(os.environ.get("ES_BUFS", "12"))
# v5 knobs
QKV_FP8 = int(os.environ.get("QKV_FP8", "0"))   # x/w in fp8e4 + DoubleRow matmuls
V_FP8 = int(os.environ.get("V_FP8", "0"))       # fp8-DR for the V matmul only
# NOTE: walrus rejects TensorScalarPtr on Pool ("Instruction engine check
# failed (Pool)"), so the rsqrt chain must stay on DVE.
RSQRT_POOL = int(os.environ.get("RSQRT_POOL", "0"))
PM_DR = mybir.MatmulPerfMode.DoubleRow
NCH = 3 if QKV_FP8 else NCHUNK  # contraction chunks for qkv matmuls
# Schraudolph kpair patterns, cycled per attention unit: "4,7|3,4,7" alternates.
SCH_PAT = [tuple(int(x) for x in grp.split(",") if x != "")
           for grp in os.environ.get("SCH_PAT", "").split("|")] \
    if os.environ.get("SCH_PAT") else [SCH_KP]
QN_BUFS = int(os.environ.get("QN_BUFS", "8"))
RS_TT = int(os.environ.get("RS_TT", "0"))   # rsqrt chain as Pool TTs w/ const tiles
CP_DVE = int(os.environ.get("CP_DVE", "0"))  # qk_sb/vA psum->sbuf copies on DVE
SCH_LAST = int(os.environ.get("SCH_LAST", "0"))  # defer sch-kpair AV to unit tail
CH2 = int(os.environ.get("CH2", "0"))        # batch rsqrt chain over tile pairs


def _bc(ap, idx, count):
    """Insert a broadcast (step 0) free dim at position idx of an AP."""
    a = list(ap.ap)
    a.insert(idx, [0, count])
    return bass.AP(tensor=ap.tensor, offset=ap.offset, ap=a)


def build_program():
    nc = bacc.Bacc(None, target_bir_lowering=False)

    qkv_dt = dt.float8e4 if QKV_FP8 else dt.bfloat16
    xT = nc.dram_tensor("xT", [C, N], qkv_dt, kind="ExternalInput")
    # host layout: [q0|k0 (256) | v (384) | q1|k1 | q2|k2]
    wqkvT = nc.dram_tensor("wqkvT", [C, 3 * HPC * D], qkv_dt, kind="ExternalInput")
    projT = nc.dram_tensor("projT", [HPC * D, C], dt.bfloat16, kind="ExternalInput")
    cqk = nc.dram_tensor("cqk", [N, 2 * D], dt.bfloat16, kind="ExternalInput")
    sqk = nc.dram_tensor("sqk", [N, 2 * D], dt.bfloat16, kind="ExternalInput")
    out = nc.dram_tensor("out", [N, C], dt.float32, kind="ExternalOutput")

    with tile.TileContext(nc) as tc:
        with (
            tc.tile_pool(name="persist", bufs=1) as persist,
            tc.tile_pool(name="qkrot", bufs=2) as qkrot,     # qT/kT rotate across pairs
            tc.tile_pool(name="work", bufs=3) as work,
            tc.tile_pool(name="qkblk", bufs=2) as qkblk,
            tc.tile_pool(name="tiny", bufs=2) as tiny,
            tc.tile_pool(name="den", bufs=2) as den,
            tc.tile_pool(name="p2e", bufs=ES_BUFS) as p2e,
            tc.tile_pool(name="outp", bufs=4) as outp,
            tc.tile_pool(name="psA", bufs=2, space="PSUM") as psA,   # qkv/tp/proj
            tc.tile_pool(name="psS", bufs=2, space="PSUM") as psS,   # scores
            tc.tile_pool(name="psV", bufs=2, space="PSUM") as psV,   # av + oT transposes
        ):
            # ---------------- persistent tiles --------------------------------
            oT = [[persist.tile([128, G], dt.bfloat16, name=f"oT{p}_{g}", tag=f"oT{p}_{g}")
                   for g in range(NG)] for p in range(3)]
            vA = [persist.tile([128, 4, HPC, D + 1], dt.bfloat16, name=f"vA{kg}", tag=f"vA{kg}")
                  for kg in range(NG)]
            identb = persist.tile([128, 128], dt.bfloat16, tag="identb")
            make_identity(nc, identb[:])
            ones1 = persist.tile([128, 1], dt.float32, tag="ones1")
            nc.vector.memset(ones1[:], 1.0)
            for kg in range(NG):
                nc.vector.tensor_copy(vA[kg][:, :, :, D : D + 1], _bc(_bc(ones1[:], 1, 4), 2, HPC))
            # broadcast constants for the Pool-TT rsqrt chain (RS_TT)
            rsc = {}
            if RS_TT:
                for nm, val, cdt in (("one_i", 1, dt.int32), ("magic", 0x5F3759DF, dt.int32),
                                     ("halfc", 0.5, dt.float32), ("c15", 1.5, dt.float32)):
                    t_c = persist.tile([128, 1], cdt, tag=f"rsc_{nm}")
                    nc.vector.memset(t_c[:], val)
                    rsc[nm] = t_c

            # weights / x^T / tables.  DMA order tuned so the first prep tiles
            # wait for the minimum byte set.
            xw_cm = tc.tile_pool(name="xw", bufs=1)
            xw = xw_cm.__enter__()
            CPC = C // NCH  # contraction rows per chunk (256 fp8-DR / 128 bf16)
            wrA = []
            wrB = []

            def _wsl(dram, j, lo, hi):
                sl = dram[j * CPC : (j + 1) * CPC, lo:hi]
                if QKV_FP8:
                    sl = sl.rearrange("(i p) f -> p i f", i=2)
                return sl

            xshp = [128, 2, G] if QKV_FP8 else [128, G]
            xr = [[xw.tile(list(xshp), qkv_dt, name=f"xr{j}_{tg}", tag=f"xr{j}_{tg}")
                   for tg in range(NG)] for j in range(NCH)]
            # interleave weight/x DMAs so chunk j's operands land together and
            # the first prep matmuls can start as early as possible
            for j in range(NCH):
                shp = [128, 2, 640] if QKV_FP8 else [128, 640]
                wa = xw.tile(shp, qkv_dt, name=f"wrA{j}", tag=f"wrA{j}")
                nc.sync.dma_start(wa[:], _wsl(wqkvT, j, 0, 640))
                wrA.append(wa)
                nc.sync.dma_start(xr[j][0][:], _wsl(xT, j, 0, G))
            tabs = {}
            for name, dram in (("cqk", cqk), ("sqk", sqk)):
                t = persist.tile([128, NT, 2, D], dt.bfloat16, name=name, tag=name)
                nc.sync.dma_start(t[:], dram.rearrange("(t p) (qk d) -> p t qk d", p=128, qk=2))
                tabs[name] = t
            for tg in range(1, NG):
                for j in range(NCH):
                    nc.sync.dma_start(xr[j][tg][:], _wsl(xT, j, tg * G, (tg + 1) * G))
            for j in range(NCH):
                shp = [128, 2, 512] if QKV_FP8 else [128, 512]
                wb = xw.tile(shp, qkv_dt, name=f"wrB{j}", tag=f"wrB{j}")
                nc.sync.dma_start(wb[:], _wsl(wqkvT, j, 640, 1152))
                wrB.append(wb)
            # V-only fp8: dedicated fp8 copies of x and the v weight columns,
            # used only by the V DoubleRow matmuls (q/k stay bf16).
            xr8, wrV = [], []
            if V_FP8:
                xT8 = nc.dram_tensor("xT8", [C, N], dt.float8e4, kind="ExternalInput")
                wv8 = nc.dram_tensor("wv8", [C, HPC * D], dt.float8e4, kind="ExternalInput")
                for j in range(3):
                    wv_ = xw.tile([128, 2, HPC * D], dt.float8e4, name=f"wrV{j}", tag=f"wrV{j}")
                    nc.sync.dma_start(wv_[:], wv8[j * 256 : (j + 1) * 256, :].rearrange("(i p) f -> p i f", i=2))
                    wrV.append(wv_)
                xr8 = [[xw.tile([128, 2, G], dt.float8e4, name=f"xr8_{j}_{tg}", tag=f"xr8_{j}_{tg}")
                        for tg in range(NG)] for j in range(3)]
                for tg in range(NG):
                    for j in range(3):
                        nc.sync.dma_start(xr8[j][tg][:], xT8[j * 256 : (j + 1) * 256, tg * G : (tg + 1) * G].rearrange("(i p) t -> p i t", i=2))
            prW = []
            for p in range(3):
                wp = persist.tile([128, C], dt.bfloat16, name=f"prW{p}", tag=f"prW{p}")
                nc.sync.dma_start(wp[:], projT[p * 128 : (p + 1) * 128, :])
                prW.append(wp)

            # ------- interleaved emission: prep / attention / projection ------
            # Engines execute their instruction streams in order, so emission
            # order IS the schedule.

            def new_pair_state(p):
                # qkT[g]: columns 0:G hold q^T for qi-group g, G:2G hold k^T
                # for ki-group g.  bf16.
                return {
                    "p": p,
                    "qkT": [qkrot.tile([128, 2 * G], dt.bfloat16, name=f"qkT{p}_{g}", tag=f"qkT{g}") for g in range(NG)],
                    "pend": [],
                    "next": 0,
                }

            def flush_one(st):
                i, qn = st["pend"].pop(0)
                # both bf16 transposes land in one psum tile -> single 2x copy
                tp = psA.tile([128, 256], dt.bfloat16, tag="qkv")
                nc.tensor.transpose(tp[:, 0:128], qn[:, 0:128], identb[:])
                nc.tensor.transpose(tp[:, 128:256], qn[:, 128:256], identb[:])
                dst = st["qkT"][i // NG][:, :].rearrange("p (qk c) -> p qk c", qk=2)[
                    :, :, (i % NG) * 128 : (i % NG + 1) * 128]
                src_v = tp[:, :].rearrange("p (qk c) -> p qk c", qk=2)
                nc.vector.tensor_copy(dst, src_v)

            _pm = PM_DR if QKV_FP8 else None
            _S1Q = 1 if QKV_FP8 else 3  # qk chunks emitted in s1

            def _xsl(j, i):
                x_t = xr[j][i // NG]
                lo, hi = (i % NG) * 128, (i % NG + 1) * 128
                return x_t[:, :, lo:hi] if QKV_FP8 else x_t[:, lo:hi]

            def _wsel(p, j, lo, hi):
                # p==0: q0|k0 in wrA[:, 0:256); p>0: qp|kp in wrB[:, (p-1)*256:p*256)
                wt = wrA[j] if p == 0 else wrB[j]
                if p != 0:
                    lo, hi = (p - 1) * 256, p * 256
                return wt[:, :, lo:hi] if QKV_FP8 else wt[:, lo:hi]

            def prep_steps(st):
                # Split one prep tile into two PE bursts so the filler can
                # spread them between attention kpairs.
                p = st["p"]
                i = st["next"]
                st["next"] += 1
                hold = {}

                def s1():
                    if p == 0:
                        vp = psA.tile([128, HPC * D], dt.float32, tag="qkv")
                        if V_FP8:
                            for j in range(3):
                                nc.tensor.matmul(vp[:], xr8[j][i // NG][:, :, (i % NG) * 128 : (i % NG + 1) * 128],
                                                 wrV[j][:], start=(j == 0), stop=(j == 2),
                                                 perf_mode=PM_DR)
                        else:
                            for j in range(NCH):
                                wsl = wrA[j][:, :, 256:640] if QKV_FP8 else wrA[j][:, 256:640]
                                nc.tensor.matmul(vp[:], _xsl(j, i), wsl,
                                                 start=(j == 0), stop=(j == NCH - 1),
                                                 perf_mode=_pm)
                        if CP_DVE:
                            nc.vector.tensor_copy(vA[i // NG][:, i % NG, :, 0:D], vp[:].rearrange("p (h d) -> p h d", h=HPC))
                        else:
                            nc.scalar.copy(vA[i // NG][:, i % NG, :, 0:D], vp[:].rearrange("p (h d) -> p h d", h=HPC))
                    qkp = psA.tile([128, 256], dt.float32, tag="qkv")
                    for j in range(_S1Q):
                        nc.tensor.matmul(qkp[:], _xsl(j, i), _wsel(p, j, 0, 256),
                                         start=(j == 0), stop=False, perf_mode=_pm)
                    hold["qkp"] = qkp

                def s2():
                    _finish_prep_tile(st, i, hold["qkp"])

                return [s1, s2]

            def _finish_prep_tile(st, i, qkp):
                p = st["p"]
                for j in range(_S1Q, NCH):
                    nc.tensor.matmul(qkp[:], _xsl(j, i), _wsel(p, j, 0, 256),
                                     start=False, stop=(j == NCH - 1), perf_mode=_pm)
                if len(st["pend"]) >= FLUSH_LAG:
                    flush_one(st)
                qk_sb = qkblk.tile([128, 256], dt.bfloat16, tag="qk_sb")
                if CP_DVE:
                    nc.vector.tensor_copy(qk_sb[:], qkp[:])
                else:
                    nc.scalar.copy(qk_sb[:], qkp[:])
                qk4 = qk_sb[:].rearrange("p (h d) -> p h d", h=4)
                # sum of squares per (token, slot).  During the pair-0 ramp
                # (no exps yet) the otherwise-idle ACT computes the squares
                # and DVE the rope add, halving Pool's prep throughput limit.
                ramp = (p == 0 and i < RAMP_N)
                sq = work.tile([128, 4, D], dt.bfloat16, tag="sq")
                if ramp:
                    nc.scalar.square(sq[:], qk4)
                else:
                    nc.gpsimd.tensor_tensor(sq[:], qk4, qk4, op=ALU.mult)
                if CH2:
                    if i % 2 == 0:
                        ss2 = tiny.tile([128, 2, 4], dt.float32, tag="ss16")
                        st["ss2"] = ss2
                        st["half"] = (i, qk4, ramp)
                        nc.vector.tensor_reduce(ss2[:, 0], sq[:], axis=AX.X, op=ALU.add)
                        return
                    ss2 = st["ss2"]
                    nc.vector.tensor_reduce(ss2[:, 1], sq[:], axis=AX.X, op=ALU.add)
                    nf8 = tiny.tile([128, 2, 4], dt.float32, tag="nf16")
                    _chain(ss2[:, :, :], nf8[:, :, :], [2, 4], ramp)
                    i0, qk40, ramp0 = st["half"]
                    _rope(st, i0, qk40, nf8[:, 0], ramp0)
                    _rope(st, i, qk4, nf8[:, 1], ramp)
                else:
                    ss = tiny.tile([128, 4], dt.float32, tag="ss16")
                    nc.vector.tensor_reduce(ss[:], sq[:], axis=AX.X, op=ALU.add)
                    nf16 = tiny.tile([128, 4], dt.float32, tag="nf16")
                    _chain(ss[:], nf16[:], [4], ramp)
                    _rope(st, i, qk4, nf16[:], ramp)

            def _chain(ss, nf16, tail, ramp):
                # rsqrt (bit-trick + Newton) on DVE: nf = 1/sqrt(ss+D*EPS)
                shape = [128] + list(tail)
                rs = nc.vector
                ssh = tiny.tile(shape, dt.float32, tag="ssh")
                rs.tensor_scalar(ssh[:], ss, 0.5, 0.5 * D * EPS,
                                 op0=ALU.mult, op1=ALU.add)
                y0i = tiny.tile(shape, dt.int32, tag="y0i")
                rs.tensor_scalar(y0i[:], ss.bitcast(dt.int32), 1, 0,
                                 op0=ALU.logical_shift_right, op1=ALU.bitwise_or)
                rs.tensor_scalar(y0i[:], y0i[:], -1, 0x5F3759DF,
                                 op0=ALU.mult, op1=ALU.add)
                yw = tiny.tile(shape, dt.float32, tag="yw")
                y = y0i[:].bitcast(dt.float32)
                for dst_ in (nf16,) if ONE_NR else (tiny.tile(shape, dt.float32, tag="y1")[:], nf16):
                    rs.tensor_tensor(yw[:], y, y, op=ALU.mult)
                    rs.tensor_tensor(yw[:], yw[:], ssh[:], op=ALU.mult)
                    rs.tensor_scalar(yw[:], yw[:], -1.0, 1.5,
                                     op0=ALU.mult, op1=ALU.add)
                    rs.tensor_tensor(dst_, y, yw[:], op=ALU.mult)
                    y = dst_

            def _rope(st, i, qk4, nf16, ramp):
                nfb = _bc(nf16, 2, D)
                t_ = work.tile([128, 4, D], dt.bfloat16, tag="t_")
                nc.gpsimd.tensor_tensor(t_[:], qk4, nfb, op=ALU.mult)
                # rope tables: [128, NT, 2(qk), D] bf16 with heads broadcast
                cwb = _bc(tabs["cqk"][:, i, :, :], 2, 2)
                swb = _bc(tabs["sqk"][:, i, :, :], 2, 2)
                t4 = t_[:].rearrange("p (qk h) d -> p qk h d", qk=2)
                h_ = D // 2
                m1 = work.tile([128, 2, 2, D], dt.bfloat16, tag="m1")
                nc.vector.tensor_tensor(m1[:], t4, cwb, op=ALU.mult)
                m2 = work.tile([128, 2, 2, D], dt.bfloat16, tag="m2")
                m2eng = nc.gpsimd if M2_POOL else nc.vector
                m2eng.tensor_tensor(m2[:, :, :, 0:h_], t4[:, :, :, h_:D], swb[:, :, :, 0:h_], op=ALU.mult)
                m2eng.tensor_tensor(m2[:, :, :, h_:D], t4[:, :, :, 0:h_], swb[:, :, :, h_:D], op=ALU.mult)
                qn = work.tile([128, 256], dt.bfloat16, tag="qn", bufs=QN_BUFS)
                qn_eng = nc.vector if ramp else nc.gpsimd
                qn_eng.tensor_tensor(qn[:].rearrange("p (qk h d) -> p qk h d", qk=2, h=2), m1[:], m2[:], op=ALU.add)
                st["pend"].append((i, qn))

            def emit_prep_tile(st):
                for s in prep_steps(st):
                    s()

            def finish_prep(st, step_q):
                while step_q:
                    step_q.pop(0)()
                while st["next"] < NT:
                    emit_prep_tile(st)
                while st["pend"]:
                    flush_one(st)

            unit_no = [0]

            def att_unit_gen(st, g, hh, oTps_box, tail_q):
                p = st["p"]
                h = 2 * p + hh
                off = 64 * hh
                sch_set = SCH_PAT[unit_no[0] % len(SCH_PAT)]
                unit_no[0] += 1
                # Two accumulation banks ping-pong across the 4 q-tiles: a
                # matmul start zeroes its whole 2KB psum bank, so concurrent
                # groups must sit in different banks.  qt0/qt1 stream with the
                # exps; qt2/qt3 re-use the banks at the unit end.
                av01 = [psV.tile([128, D + 1], dt.float32, name=f"av{qt}", tag="av",
                                 padded_shape=[128, 512]) for qt in range(2)]

                def emit_av(kpair, es, is_i16, qts, avt):
                    for half in range(2):
                        ki = kpair * 2 + half
                        for j, qt in enumerate(qts):
                            esb = es[:, half * 512 + qt * 128 : half * 512 + (qt + 1) * 128]
                            if is_i16:
                                esb = esb.bitcast(dt.bfloat16)
                            nc.tensor.matmul(
                                avt[j][:],
                                esb,
                                vA[ki // NG][:, ki % NG, h, :],
                                start=(ki == 0), stop=(ki == NT - 1),
                            )

                rd4 = den.tile([128, 4], dt.float32, tag="rd4")
                o_sb = den.tile([128, 4, D], dt.bfloat16, tag="o_sb")

                def normalize(qts, avt):
                    for j, qt in enumerate(qts):
                        nc.vector.reciprocal(rd4[:, qt : qt + 1], avt[j][:, D : D + 1])
                        nc.vector.tensor_scalar(o_sb[:, qt, :], avt[j][:, 0:D],
                                                rd4[:, qt : qt + 1], None, op0=ALU.mult)

                pend = []
                all_es = []
                for kpair in range(8):
                    sp = psS.tile([128, 1024], dt.float32, tag="sp")
                    for half in range(2):
                        ki = kpair * 2 + half
                        nc.tensor.matmul(
                            sp[:, half * 512 : (half + 1) * 512],
                            st["qkT"][ki // NG][off : off + 64, G + (ki % NG) * 128 : G + (ki % NG + 1) * 128],
                            st["qkT"][g][off : off + 64, 0:G],
                            start=True, stop=True,
                        )
                    # exp: mostly on ACT; SCH_KP kpairs on DVE via the bf16
                    # Schraudolph bit-trick (one tensor_scalar; GPSIMD cannot
                    # read PSUM so Pool is out).
                    if kpair in sch_set:
                        esi = p2e.tile([128, 1024], dt.int16, name="esi", tag="es")
                        nc.vector.tensor_scalar(esi[:], sp[:], SCH_C0, SCH_C1,
                                                op0=ALU.mult, op1=ALU.add)
                        entry = (kpair, esi, True)
                    else:
                        es = p2e.tile([128, 1024], dt.bfloat16, name="est", tag="es")
                        nc.scalar.activation(es[:], sp[:], AF.Exp, scale=SCALE)
                        entry = (kpair, es, False)
                    pend.append(entry)
                    all_es.append(entry)
                    if len(pend) > AV_TRAIL:
                        # prefer draining ACT-exp'd tiles; Schraudolph tiles
                        # (DVE, often late) defer to the unit tail so the PE
                        # never waits on them mid-unit (SCH_LAST).
                        idx = 0
                        if SCH_LAST:
                            for ei, e_ in enumerate(pend):
                                if not e_[2]:
                                    idx = ei
                                    break
                        e = pend.pop(idx)
                        emit_av(e[0], e[1], e[2], (0, 1), av01)
                    yield
                # The whole unit tail — qt0/qt1 drain, qt2/qt3 accumulation
                # and the feature-major transposes — is deferred into the NEXT
                # unit, staged after its first scores, so the PE never sits
                # behind exp(kp7) at a unit boundary.
                av23 = [psV.tile([128, D + 1], dt.float32, name=f"av{qt}", tag="av",
                                 padded_shape=[128, 512]) for qt in (2, 3)]

                def tail_av01():
                    for e in pend:
                        emit_av(e[0], e[1], e[2], (0, 1), av01)
                    normalize((0, 1), av01)

                def tail_av23():
                    for e in all_es:
                        emit_av(e[0], e[1], e[2], (2, 3), av23)
                    normalize((2, 3), av23)

                def tail_tp():
                    oTps = psA.tile([128, 4, 128], dt.bfloat16, name="oTps", tag="qkv")
                    for qt in range(4):
                        nc.tensor.transpose(oTps[off : off + 64, qt, :], o_sb[:, qt, :], identb[:])
                    nc.vector.tensor_copy(
                        oT[p][g][:, :].rearrange("p (qt c) -> p qt c", qt=4)[off : off + 64],
                        oTps[off : off + 64])
                tail_q.append((tail_av01, tail_av23, tail_tp))

            tail_q = []

            def drain_tails():
                while tail_q:
                    for f in tail_q.pop(0):
                        f()

            def drive_unit(st, g, hh, box, filler=None):
                # filler(kp) runs at every kpair boundary so prep/proj PE work
                # spreads inside the unit instead of bunching at its end; the
                # previous unit's deferred qt2/qt3 accumulation runs after this
                # unit's kp1 scores, its transposes one kpair later.
                stages = []
                for kp, _ in enumerate(att_unit_gen(st, g, hh, box, tail_q)):
                    if kp == 1:
                        while len(tail_q) > 1:
                            for f in tail_q.pop(0):
                                f()
                        if tail_q:
                            stages = list(tail_q.pop(0))
                    elif kp in (2, 3) and stages:
                        stages.pop(0)()
                    if kp == 1 and stages:
                        stages.pop(0)()
                    if filler is not None:
                        filler(kp)
                for f in stages:
                    f()

            def proj_steps(i):
                hold = {}

                def s1():
                    p512 = psA.tile([128, 512], dt.float32, tag="qkv")
                    for pp_ in range(3):
                        sl = oT[pp_][i // NG][:, (i % NG) * 128 : (i % NG + 1) * 128]
                        nc.tensor.matmul(p512[:], sl, prW[pp_][:, 0:512],
                                         start=(pp_ == 0), stop=(pp_ == 2))
                    os_ = outp.tile([128, C], dt.float32, tag="os")
                    nc.vector.tensor_copy(os_[:, 0:512], p512[:])
                    hold["os"] = os_

                def s2():
                    p256 = psA.tile([128, 256], dt.float32, tag="qkv")
                    for pp_ in range(3):
                        sl = oT[pp_][i // NG][:, (i % NG) * 128 : (i % NG + 1) * 128]
                        nc.tensor.matmul(p256[:], sl, prW[pp_][:, 512:768],
                                         start=(pp_ == 0), stop=(pp_ == 2))
                    os_ = hold["os"]
                    nc.vector.tensor_copy(os_[:, 512:768], p256[:])
                    nc.sync.dma_start(out[i * 128 : (i + 1) * 128, :], os_[:])

                return [s1, s2]

            def emit_proj_tile(i):
                for s in proj_steps(i):
                    s()

            # pair-0 prep up front (V matmuls included); the first attention
            # unit's kpairs are interleaved as soon as their kT/vA quads are
            # flushed, so the ACT exp stream starts early.
            cur = new_pair_state(0)
            box0 = {}
            gen0 = att_unit_gen(cur, 0, 0, box0, tail_q)
            gate = [max(3, 2 * j + 1) + FLUSH_LAG for j in range(8)]
            gate = [g if g <= NT - 1 else 99 for g in gate]
            kp_done = 0
            for i in range(NT):
                emit_prep_tile(cur)
                while kp_done < 8 and i >= gate[kp_done]:
                    next(gen0)
                    kp_done += 1
            finish_prep(cur, [])
            for _ in gen0:
                pass

            proj_queue = list(range(NT))
            for p in range(3):
                nxt = new_pair_state(p + 1) if p < 2 else None
                step_q = []

                def filler(g_cur, _nxt=nxt, _sq=step_q):
                    def f(kp):
                        if kp not in (FILL_KP if _nxt is not None else PROJ_KP):
                            return
                        if _nxt is not None:
                            if not _sq and _nxt["next"] < NT:
                                _sq.extend(prep_steps(_nxt))
                            if _sq:
                                _sq.pop(0)()
                        else:
                            if not _sq and proj_queue and proj_queue[0] < g_cur * NG:
                                _sq.extend(proj_steps(proj_queue.pop(0)))
                            if _sq:
                                _sq.pop(0)()
                    return f

                for g in range(NG):
                    box = box0 if (p == 0 and g == 0) else {}
                    for hh in range(2):
                        if p == 0 and g == 0 and hh == 0:
                            continue
                        drive_unit(cur, g, hh, box, filler(g))
                if nxt is not None:
                    finish_prep(nxt, step_q)
                    cur = nxt
                else:
                    while step_q:
                        step_q.pop(0)()
            drain_tails()
            last_steps = [proj_steps(i) for i in proj_queue]
            for s_idx in range(2):
                for ss in last_steps:
                    ss[s_idx]()
            xw_cm.__exit__(None, None, None)

    nc.compile()
    return nc


_NC = None


def _get_nc():
    global _NC
    if _NC is None:
        _NC = build_program()
    return _NC


def qkv_np_dt():
    import ml_dtypes
    return ml_dtypes.float8_e4m3 if QKV_FP8 else ml_dtypes.bfloat16


def _prep_inputs(x, cos, sin, qkv_w, q_norm_w, k_norm_w, proj_w):
    import ml_dtypes
    bf16 = ml_dtypes.bfloat16
    cos2 = np.asarray(cos, np.float32).reshape(N, D // 2)
    sin2 = np.asarray(sin, np.float32).reshape(N, D // 2)
    cos_full = np.concatenate([cos2, cos2], axis=1)          # [N, 64]
    sin_signed = np.concatenate([-sin2, sin2], axis=1)       # [N, 64]

    def tables(w):
        w = np.asarray(w, np.float32)
        wswap = np.concatenate([w[D // 2 :], w[: D // 2]])
        cw = (8.0 * cos_full * w[None, :]).astype(np.float32)
        sw = (8.0 * sin_signed * wswap[None, :]).astype(np.float32)
        return np.ascontiguousarray(cw), np.ascontiguousarray(sw)

    cwq_, swq_ = tables(q_norm_w)
    cwk_, swk_ = tables(k_norm_w)
    cqk_ = np.ascontiguousarray(np.stack([cwq_, cwk_], axis=1).reshape(N, 2 * D)).astype(bf16)
    sqk_ = np.ascontiguousarray(np.stack([swq_, swk_], axis=1).reshape(N, 2 * D)).astype(bf16)

    in_maps = []
    for c in range(8):
        b, hg = c // 2, c % 2
        h0 = HPC * hg
        rows = np.r_[h0 * D : (h0 + HPC) * D]
        wq = qkv_w[rows]          # [384, C]
        wk = qkv_w[C + rows]
        wv = qkv_w[2 * C + rows]
        # pack as [q0|k0 (256), v (384), q1|k1, q2|k2]
        parts = [wq[0:128], wk[0:128], wv]
        for p in range(1, 3):
            parts.append(wq[p * 128 : (p + 1) * 128])
            parts.append(wk[p * 128 : (p + 1) * 128])
        qdt = qkv_np_dt()
        wqkvT_ = np.ascontiguousarray(np.concatenate(parts, 0).T).astype(qdt)
        projT_ = np.ascontiguousarray(proj_w[:, rows].T).astype(bf16)
        xT_ = np.ascontiguousarray(x[b].T).astype(qdt)
        m = {
            "xT": xT_, "wqkvT": wqkvT_, "projT": projT_,
            "cqk": cqk_, "sqk": sqk_,
        }
        if V_FP8:
            e4 = ml_dtypes.float8_e4m3
            m["xT8"] = np.ascontiguousarray(x[b].T).astype(e4)
            m["wv8"] = np.ascontiguousarray(wv.T).astype(e4)
        in_maps.append(m)
    return in_maps


def kernel(x, cos, sin, qkv_w, q_norm_w, k_norm_w, proj_w, proj_b, _want_trace=False):
    x = np.asarray(x, np.float32)
    qkv_w = np.asarray(qkv_w, np.float32)
    proj_w = np.asarray(proj_w, np.float32)
    proj_b = np.asarray(proj_b, np.float32)
    in_maps = _prep_inputs(x, cos, sin, qkv_w, q_norm_w, k_norm_w, proj_w)
    nc = _get_nc()
    res = run_bass_kernel_spmd(nc, in_maps, core_ids=list(range(8)), trace=_want_trace)
    out = np.empty((B, N, C), np.float32)
    for b in range(B):
        out[b] = res.results[2 * b]["out"] + res.results[2 * b + 1]["out"] + proj_b[None, :]
    if _want_trace:
        return out, res
    return out



# revision 30
# speedup vs baseline: 1.0240x; 1.0091x over previous
"""Trainium2 Bass kernel for nn_Attention (B=4, N=2048, C=768, H=12, D=64).

Sharding: core c -> batch b=c//2, head-group hg=c%2 (6 heads each).
qkv_w column-parallel, proj_w row-parallel (host sums the 2 partials per b).

v4 structure (vs the f32r baseline):
  - QKV matmuls stay f32r (full rate, moving dims >= 256).
  - q/k tiles, rope tables, exp'd scores (es), V, attention output and the
    projection all run in bf16: transposes cost 1.0 cycles/row and the
    psum<->sbuf copies hit the DVE 2x half-word mode.
  - AV is FLIPPED: es [k,q] is the stationary operand, [v|1] the moving one,
    so each 128x128 score block costs 65 PE rows instead of 512. The ones
    column gives the softmax denominator in psum column 64. Output is
    token-major [q, d]; a bf16 PE transpose + copy rebuilds feature-major oT
    for the projection.
  - A matmul `start` zeroes its whole 2KB psum bank, so the 4 q-subtiles of a
    unit accumulate as qt0/qt1 in two ping-pong banks streaming with the
    exps, then qt2/qt3 re-use those banks; the qt0/1 drain, qt2/3
    accumulation and the transposes are deferred into the NEXT unit (staged
    after its first scores) so the PE never idles behind exp(kp7).
  - exp runs on ACT except 2 of 8 kpairs per unit, which run on DVE as ONE
    tensor_scalar: i16 = trunc(s*c0 + c1) bitcast to bf16 is a Schraudolph
    exp (max rel err ~3% on those tiles; end-to-end ~9e-3 vs 2e-2 budget).
  - GPSIMD cannot touch PSUM: Pool only carries sbuf-side rope math
    (sq/t_/qn); all psum exits live on DVE/ACT.
  - rsqrt via bit-trick seed + 1 Newton step (0.2% worst-case, q-side
    cancels in softmax).
  - No max-subtraction needed: RMSNorm bounds the logits (|z| <= ~16).
"""
import sys

sys.path.insert(0, "/opt/trn_rl_repo")

import numpy as np
import concourse.bass as bass
import concourse.mybir as mybir
import concourse.tile as tile
from concourse import bacc
from concourse.bass_utils import run_bass_kernel_spmd
from concourse.masks import make_identity

dt = mybir.dt
AF = mybir.ActivationFunctionType
ALU = mybir.AluOpType
AX = mybir.AxisListType

B, N, C = 4, 2048, 768
H, D = 12, 64
HPC = 6            # heads per core
EPS = 1e-6
NT = N // 128      # 16 token tiles
NCHUNK = C // 128  # 6 contraction chunks
SCALE = D ** -0.5  # 0.125
NG = 4             # qi groups
G = N // NG        # 512 per group
# bf16 Schraudolph exp: i16 = trunc(s*SCH_C0 + SCH_C1); bitcast bf16 ~ exp(s/8)
SCH_C0 = 184.664965 * SCALE
SCH_C1 = 16250.5
import os
# kpair indices whose exp runs on DVE via Schraudolph ("" = none)
SCH_KP = tuple(int(x) for x in os.environ.get("SCH_KP", "5,7").split(",") if x != "")
FILL_KP = tuple(int(x) for x in os.environ.get("FILL_KP", "0,2,5,6").split(","))
AV_TRAIL = int(os.environ.get("AV_TRAIL", "5"))
PROJ_KP = tuple(int(x) for x in os.environ.get("PROJ_KP", "4,5,6,7").split(","))
ONE_NR = int(os.environ.get("ONE_NR", "1"))
M2_POOL = int(os.environ.get("M2_POOL", "0"))
SCH_DEFER = int(os.environ.get("SCH_DEFER", "0"))
FLUSH_LAG = int(os.environ.get("FLUSH_LAG", "10"))
TAIL_KP = int(os.environ.get("TAIL_KP", "0"))
RAMP_N = int(os.environ.get("RAMP_N", "16"))
ES_BUFS = int(os.environ.get("ES_BUFS", "12"))
# v5 knobs
QKV_FP8 = int(os.environ.get("QKV_FP8", "0"))   # x/w in fp8e4 + DoubleRow matmuls
V_FP8 = int(os.environ.get("V_FP8", "0"))       # fp8-DR for the V matmul only
# NOTE: walrus rejects TensorScalarPtr on Pool ("Instruction engine check
# failed (Pool)"), so the rsqrt chain must stay on DVE.
RSQRT_POOL = int(os.environ.get("RSQRT_POOL", "0"))
PM_DR = mybir.MatmulPerfMode.DoubleRow
NCH = 3 if QKV_FP8 else NCHUNK  # contraction chunks for qkv matmuls
# Schraudolph kpair patterns, cycled per attention unit: "4,7|3,4,7" alternates.
SCH_PAT = [tuple(int(x) for x in grp.split(",") if x != "")
           for grp in os.environ.get("SCH_PAT", "").split("|")] \
    if os.environ.get("SCH_PAT") else [SCH_KP]
QN_BUFS = int(os.environ.get("QN_BUFS", "20"))
RS_TT = int(os.environ.get("RS_TT", "0"))   # rsqrt chain as Pool TTs w/ const tiles
CP_DVE = int(os.environ.get("CP_DVE", "0"))  # qk_sb/vA psum->sbuf copies on DVE
SCH_LAST = int(os.environ.get("SCH_LAST", "0"))  # defer sch-kpair AV to unit tail
CH2 = int(os.environ.get("CH2", "0"))        # batch rsqrt chain over tile pairs


def _bc(ap, idx, count):
    """Insert a broadcast (step 0) free dim at position idx of an AP."""
    a = list(ap.ap)
    a.insert(idx, [0, count])
    return bass.AP(tensor=ap.tensor, offset=ap.offset, ap=a)


def build_program():
    nc = bacc.Bacc(None, target_bir_lowering=False)

    qkv_dt = dt.float8e4 if QKV_FP8 else dt.bfloat16
    xT = nc.dram_tensor("xT", [C, N], qkv_dt, kind="ExternalInput")
    # host layout: [q0|k0 (256) | v (384) | q1|k1 | q2|k2]
    wqkvT = nc.dram_tensor("wqkvT", [C, 3 * HPC * D], qkv_dt, kind="ExternalInput")
    projT = nc.dram_tensor("projT", [HPC * D, C], dt.bfloat16, kind="ExternalInput")
    cqk = nc.dram_tensor("cqk", [N, 2 * D], dt.bfloat16, kind="ExternalInput")
    sqk = nc.dram_tensor("sqk", [N, 2 * D], dt.bfloat16, kind="ExternalInput")
    out = nc.dram_tensor("out", [N, C], dt.float32, kind="ExternalOutput")

    with tile.TileContext(nc) as tc:
        with (
            tc.tile_pool(name="persist", bufs=1) as persist,
            tc.tile_pool(name="qkrot", bufs=2) as qkrot,     # qT/kT rotate across pairs
            tc.tile_pool(name="work", bufs=3) as work,
            tc.tile_pool(name="qkblk", bufs=2) as qkblk,
            tc.tile_pool(name="tiny", bufs=2) as tiny,
            tc.tile_pool(name="den", bufs=2) as den,
            tc.tile_pool(name="p2e", bufs=ES_BUFS) as p2e,
            tc.tile_pool(name="outp", bufs=4) as outp,
            tc.tile_pool(name="psA", bufs=2, space="PSUM") as psA,   # qkv/tp/proj
            tc.tile_pool(name="psS", bufs=2, space="PSUM") as psS,   # scores
            tc.tile_pool(name="psV", bufs=2, space="PSUM") as psV,   # av + oT transposes
        ):
            # ---------------- persistent tiles --------------------------------
            oT = [[persist.tile([128, G], dt.bfloat16, name=f"oT{p}_{g}", tag=f"oT{p}_{g}")
                   for g in range(NG)] for p in range(3)]
            vA = [persist.tile([128, 4, HPC, D + 1], dt.bfloat16, name=f"vA{kg}", tag=f"vA{kg}")
                  for kg in range(NG)]
            identb = persist.tile([128, 128], dt.bfloat16, tag="identb")
            make_identity(nc, identb[:])
            ones1 = persist.tile([128, 1], dt.float32, tag="ones1")
            nc.vector.memset(ones1[:], 1.0)
            for kg in range(NG):
                nc.vector.tensor_copy(vA[kg][:, :, :, D : D + 1], _bc(_bc(ones1[:], 1, 4), 2, HPC))
            # broadcast constants for the Pool-TT rsqrt chain (RS_TT)
            rsc = {}
            if RS_TT:
                for nm, val, cdt in (("one_i", 1, dt.int32), ("magic", 0x5F3759DF, dt.int32),
                                     ("halfc", 0.5, dt.float32), ("c15", 1.5, dt.float32)):
                    t_c = persist.tile([128, 1], cdt, tag=f"rsc_{nm}")
                    nc.vector.memset(t_c[:], val)
                    rsc[nm] = t_c

            # weights / x^T / tables.  DMA order tuned so the first prep tiles
            # wait for the minimum byte set.
            xw_cm = tc.tile_pool(name="xw", bufs=1)
            xw = xw_cm.__enter__()
            CPC = C // NCH  # contraction rows per chunk (256 fp8-DR / 128 bf16)
            wrA = []
            wrB = []

            def _wsl(dram, j, lo, hi):
                sl = dram[j * CPC : (j + 1) * CPC, lo:hi]
                if QKV_FP8:
                    sl = sl.rearrange("(i p) f -> p i f", i=2)
                return sl

            xshp = [128, 2, G] if QKV_FP8 else [128, G]
            xr = [[xw.tile(list(xshp), qkv_dt, name=f"xr{j}_{tg}", tag=f"xr{j}_{tg}")
                   for tg in range(NG)] for j in range(NCH)]
            # interleave weight/x DMAs so chunk j's operands land together and
            # the first prep matmuls can start as early as possible
            for j in range(NCH):
                shp = [128, 2, 640] if QKV_FP8 else [128, 640]
                wa = xw.tile(shp, qkv_dt, name=f"wrA{j}", tag=f"wrA{j}")
                nc.sync.dma_start(wa[:], _wsl(wqkvT, j, 0, 640))
                wrA.append(wa)
                nc.sync.dma_start(xr[j][0][:], _wsl(xT, j, 0, G))
            tabs = {}
            for name, dram in (("cqk", cqk), ("sqk", sqk)):
                t = persist.tile([128, NT, 2, D], dt.bfloat16, name=name, tag=name)
                nc.sync.dma_start(t[:], dram.rearrange("(t p) (qk d) -> p t qk d", p=128, qk=2))
                tabs[name] = t
            for tg in range(1, NG):
                for j in range(NCH):
                    nc.sync.dma_start(xr[j][tg][:], _wsl(xT, j, tg * G, (tg + 1) * G))
            for j in range(NCH):
                shp = [128, 2, 512] if QKV_FP8 else [128, 512]
                wb = xw.tile(shp, qkv_dt, name=f"wrB{j}", tag=f"wrB{j}")
                nc.sync.dma_start(wb[:], _wsl(wqkvT, j, 640, 1152))
                wrB.append(wb)
            # V-only fp8: dedicated fp8 copies of x and the v weight columns,
            # used only by the V DoubleRow matmuls (q/k stay bf16).
            xr8, wrV = [], []
            if V_FP8:
                xT8 = nc.dram_tensor("xT8", [C, N], dt.float8e4, kind="ExternalInput")
                wv8 = nc.dram_tensor("wv8", [C, HPC * D], dt.float8e4, kind="ExternalInput")
                for j in range(3):
                    wv_ = xw.tile([128, 2, HPC * D], dt.float8e4, name=f"wrV{j}", tag=f"wrV{j}")
                    nc.sync.dma_start(wv_[:], wv8[j * 256 : (j + 1) * 256, :].rearrange("(i p) f -> p i f", i=2))
                    wrV.append(wv_)
                xr8 = [[xw.tile([128, 2, G], dt.float8e4, name=f"xr8_{j}_{tg}", tag=f"xr8_{j}_{tg}")
                        for tg in range(NG)] for j in range(3)]
                for tg in range(NG):
                    for j in range(3):
                        nc.sync.dma_start(xr8[j][tg][:], xT8[j * 256 : (j + 1) * 256, tg * G : (tg + 1) * G].rearrange("(i p) t -> p i t", i=2))
            prW = []
            for p in range(3):
                wp = persist.tile([128, C], dt.bfloat16, name=f"prW{p}", tag=f"prW{p}")
                nc.sync.dma_start(wp[:], projT[p * 128 : (p + 1) * 128, :])
                prW.append(wp)

            # ------- interleaved emission: prep / attention / projection ------
            # Engines execute their instruction streams in order, so emission
            # order IS the schedule.

            def new_pair_state(p):
                # qkT[g]: columns 0:G hold q^T for qi-group g, G:2G hold k^T
                # for ki-group g.  bf16.
                return {
                    "p": p,
                    "qkT": [qkrot.tile([128, 2 * G], dt.bfloat16, name=f"qkT{p}_{g}", tag=f"qkT{g}") for g in range(NG)],
                    "pend": [],
                    "next": 0,
                }

            def flush_one(st):
                i, qn = st["pend"].pop(0)
                # both bf16 transposes land in one psum tile -> single 2x copy
                tp = psA.tile([128, 256], dt.bfloat16, tag="qkv")
                nc.tensor.transpose(tp[:, 0:128], qn[:, 0:128], identb[:])
                nc.tensor.transpose(tp[:, 128:256], qn[:, 128:256], identb[:])
                dst = st["qkT"][i // NG][:, :].rearrange("p (qk c) -> p qk c", qk=2)[
                    :, :, (i % NG) * 128 : (i % NG + 1) * 128]
                src_v = tp[:, :].rearrange("p (qk c) -> p qk c", qk=2)
                nc.vector.tensor_copy(dst, src_v)

            _pm = PM_DR if QKV_FP8 else None
            _S1Q = 1 if QKV_FP8 else 3  # qk chunks emitted in s1

            def _xsl(j, i):
                x_t = xr[j][i // NG]
                lo, hi = (i % NG) * 128, (i % NG + 1) * 128
                return x_t[:, :, lo:hi] if QKV_FP8 else x_t[:, lo:hi]

            def _wsel(p, j, lo, hi):
                # p==0: q0|k0 in wrA[:, 0:256); p>0: qp|kp in wrB[:, (p-1)*256:p*256)
                wt = wrA[j] if p == 0 else wrB[j]
                if p != 0:
                    lo, hi = (p - 1) * 256, p * 256
                return wt[:, :, lo:hi] if QKV_FP8 else wt[:, lo:hi]

            def prep_steps(st):
                # Split one prep tile into two PE bursts so the filler can
                # spread them between attention kpairs.
                p = st["p"]
                i = st["next"]
                st["next"] += 1
                hold = {}

                def s1():
                    if p == 0:
                        vp = psA.tile([128, HPC * D], dt.float32, tag="qkv")
                        if V_FP8:
                            for j in range(3):
                                nc.tensor.matmul(vp[:], xr8[j][i // NG][:, :, (i % NG) * 128 : (i % NG + 1) * 128],
                                                 wrV[j][:], start=(j == 0), stop=(j == 2),
                                                 perf_mode=PM_DR)
                        else:
                            for j in range(NCH):
                                wsl = wrA[j][:, :, 256:640] if QKV_FP8 else wrA[j][:, 256:640]
                                nc.tensor.matmul(vp[:], _xsl(j, i), wsl,
                                                 start=(j == 0), stop=(j == NCH - 1),
                                                 perf_mode=_pm)
                        if CP_DVE:
                            nc.vector.tensor_copy(vA[i // NG][:, i % NG, :, 0:D], vp[:].rearrange("p (h d) -> p h d", h=HPC))
                        else:
                            nc.scalar.copy(vA[i // NG][:, i % NG, :, 0:D], vp[:].rearrange("p (h d) -> p h d", h=HPC))
                    qkp = psA.tile([128, 256], dt.float32, tag="qkv")
                    for j in range(_S1Q):
                        nc.tensor.matmul(qkp[:], _xsl(j, i), _wsel(p, j, 0, 256),
                                         start=(j == 0), stop=False, perf_mode=_pm)
                    hold["qkp"] = qkp

                def s2():
                    _finish_prep_tile(st, i, hold["qkp"])

                return [s1, s2]

            def _finish_prep_tile(st, i, qkp):
                p = st["p"]
                for j in range(_S1Q, NCH):
                    nc.tensor.matmul(qkp[:], _xsl(j, i), _wsel(p, j, 0, 256),
                                     start=False, stop=(j == NCH - 1), perf_mode=_pm)
                if len(st["pend"]) >= FLUSH_LAG:
                    flush_one(st)
                qk_sb = qkblk.tile([128, 256], dt.bfloat16, tag="qk_sb")
                if CP_DVE:
                    nc.vector.tensor_copy(qk_sb[:], qkp[:])
                else:
                    nc.scalar.copy(qk_sb[:], qkp[:])
                qk4 = qk_sb[:].rearrange("p (h d) -> p h d", h=4)
                # sum of squares per (token, slot).  During the pair-0 ramp
                # (no exps yet) the otherwise-idle ACT computes the squares
                # and DVE the rope add, halving Pool's prep throughput limit.
                ramp = (p == 0 and i < RAMP_N)
                sq = work.tile([128, 4, D], dt.bfloat16, tag="sq")
                if ramp:
                    nc.scalar.square(sq[:], qk4)
                else:
                    nc.gpsimd.tensor_tensor(sq[:], qk4, qk4, op=ALU.mult)
                if CH2:
                    if i % 2 == 0:
                        ss2 = tiny.tile([128, 2, 4], dt.float32, tag="ss16")
                        st["ss2"] = ss2
                        st["half"] = (i, qk4, ramp)
                        nc.vector.tensor_reduce(ss2[:, 0], sq[:], axis=AX.X, op=ALU.add)
                        return
                    ss2 = st["ss2"]
                    nc.vector.tensor_reduce(ss2[:, 1], sq[:], axis=AX.X, op=ALU.add)
                    nf8 = tiny.tile([128, 2, 4], dt.float32, tag="nf16")
                    _chain(ss2[:, :, :], nf8[:, :, :], [2, 4], ramp)
                    i0, qk40, ramp0 = st["half"]
                    _rope(st, i0, qk40, nf8[:, 0], ramp0)
                    _rope(st, i, qk4, nf8[:, 1], ramp)
                else:
                    ss = tiny.tile([128, 4], dt.float32, tag="ss16")
                    nc.vector.tensor_reduce(ss[:], sq[:], axis=AX.X, op=ALU.add)
                    nf16 = tiny.tile([128, 4], dt.float32, tag="nf16")
                    _chain(ss[:], nf16[:], [4], ramp)
                    _rope(st, i, qk4, nf16[:], ramp)

            def _chain(ss, nf16, tail, ramp):
                # rsqrt (bit-trick + Newton) on DVE: nf = 1/sqrt(ss+D*EPS)
                shape = [128] + list(tail)
                rs = nc.vector
                ssh = tiny.tile(shape, dt.float32, tag="ssh")
                rs.tensor_scalar(ssh[:], ss, 0.5, 0.5 * D * EPS,
                                 op0=ALU.mult, op1=ALU.add)
                y0i = tiny.tile(shape, dt.int32, tag="y0i")
                rs.tensor_scalar(y0i[:], ss.bitcast(dt.int32), 1, 0,
                                 op0=ALU.logical_shift_right, op1=ALU.bitwise_or)
                rs.tensor_scalar(y0i[:], y0i[:], -1, 0x5F3759DF,
                                 op0=ALU.mult, op1=ALU.add)
                yw = tiny.tile(shape, dt.float32, tag="yw")
                y = y0i[:].bitcast(dt.float32)
                for dst_ in (nf16,) if ONE_NR else (tiny.tile(shape, dt.float32, tag="y1")[:], nf16):
                    rs.tensor_tensor(yw[:], y, y, op=ALU.mult)
                    rs.tensor_tensor(yw[:], yw[:], ssh[:], op=ALU.mult)
                    rs.tensor_scalar(yw[:], yw[:], -1.0, 1.5,
                                     op0=ALU.mult, op1=ALU.add)
                    rs.tensor_tensor(dst_, y, yw[:], op=ALU.mult)
                    y = dst_

            def _rope(st, i, qk4, nf16, ramp):
                nfb = _bc(nf16, 2, D)
                t_ = work.tile([128, 4, D], dt.bfloat16, tag="t_")
                nc.gpsimd.tensor_tensor(t_[:], qk4, nfb, op=ALU.mult)
                # rope tables: [128, NT, 2(qk), D] bf16 with heads broadcast
                cwb = _bc(tabs["cqk"][:, i, :, :], 2, 2)
                swb = _bc(tabs["sqk"][:, i, :, :], 2, 2)
                t4 = t_[:].rearrange("p (qk h) d -> p qk h d", qk=2)
                h_ = D // 2
                m1 = work.tile([128, 2, 2, D], dt.bfloat16, tag="m1")
                nc.vector.tensor_tensor(m1[:], t4, cwb, op=ALU.mult)
                m2 = work.tile([128, 2, 2, D], dt.bfloat16, tag="m2")
                m2eng = nc.gpsimd if M2_POOL else nc.vector
                m2eng.tensor_tensor(m2[:, :, :, 0:h_], t4[:, :, :, h_:D], swb[:, :, :, 0:h_], op=ALU.mult)
                m2eng.tensor_tensor(m2[:, :, :, h_:D], t4[:, :, :, 0:h_], swb[:, :, :, h_:D], op=ALU.mult)
                qn = work.tile([128, 256], dt.bfloat16, tag="qn", bufs=QN_BUFS)
                qn_eng = nc.vector if ramp else nc.gpsimd
                qn_eng.tensor_tensor(qn[:].rearrange("p (qk h d) -> p qk h d", qk=2, h=2), m1[:], m2[:], op=ALU.add)
                st["pend"].append((i, qn))

            def emit_prep_tile(st):
                for s in prep_steps(st):
                    s()

            def finish_prep(st, step_q):
                while step_q:
                    step_q.pop(0)()
                while st["next"] < NT:
                    emit_prep_tile(st)
                while st["pend"]:
                    flush_one(st)

            unit_no = [0]

            def att_unit_gen(st, g, hh, oTps_box, tail_q):
                p = st["p"]
                h = 2 * p + hh
                off = 64 * hh
                sch_set = SCH_PAT[unit_no[0] % len(SCH_PAT)]
                unit_no[0] += 1
                # Two accumulation banks ping-pong across the 4 q-tiles: a
                # matmul start zeroes its whole 2KB psum bank, so concurrent
                # groups must sit in different banks.  qt0/qt1 stream with the
                # exps; qt2/qt3 re-use the banks at the unit end.
                av01 = [psV.tile([128, D + 1], dt.float32, name=f"av{qt}", tag="av",
                                 padded_shape=[128, 512]) for qt in range(2)]

                def emit_av(kpair, es, is_i16, qts, avt):
                    for half in range(2):
                        ki = kpair * 2 + half
                        for j, qt in enumerate(qts):
                            esb = es[:, half * 512 + qt * 128 : half * 512 + (qt + 1) * 128]
                            if is_i16:
                                esb = esb.bitcast(dt.bfloat16)
                            nc.tensor.matmul(
                                avt[j][:],
                                esb,
                                vA[ki // NG][:, ki % NG, h, :],
                                start=(ki == 0), stop=(ki == NT - 1),
                            )

                rd4 = den.tile([128, 4], dt.float32, tag="rd4")
                o_sb = den.tile([128, 4, D], dt.bfloat16, tag="o_sb")

                def normalize(qts, avt):
                    for j, qt in enumerate(qts):
                        nc.vector.reciprocal(rd4[:, qt : qt + 1], avt[j][:, D : D + 1])
                        nc.vector.tensor_scalar(o_sb[:, qt, :], avt[j][:, 0:D],
                                                rd4[:, qt : qt + 1], None, op0=ALU.mult)

                pend = []
                all_es = []
                for kpair in range(8):
                    sp = psS.tile([128, 1024], dt.float32, tag="sp")
                    for half in range(2):
                        ki = kpair * 2 + half
                        nc.tensor.matmul(
                            sp[:, half * 512 : (half + 1) * 512],
                            st["qkT"][ki // NG][off : off + 64, G + (ki % NG) * 128 : G + (ki % NG + 1) * 128],
                            st["qkT"][g][off : off + 64, 0:G],
                            start=True, stop=True,
                        )
                    # exp: mostly on ACT; SCH_KP kpairs on DVE via the bf16
                    # Schraudolph bit-trick (one tensor_scalar; GPSIMD cannot
                    # read PSUM so Pool is out).
                    if kpair in sch_set:
                        esi = p2e.tile([128, 1024], dt.int16, name="esi", tag="es")
                        nc.vector.tensor_scalar(esi[:], sp[:], SCH_C0, SCH_C1,
                                                op0=ALU.mult, op1=ALU.add)
                        entry = (kpair, esi, True)
                    else:
                        es = p2e.tile([128, 1024], dt.bfloat16, name="est", tag="es")
                        nc.scalar.activation(es[:], sp[:], AF.Exp, scale=SCALE)
                        entry = (kpair, es, False)
                    pend.append(entry)
                    all_es.append(entry)
                    if len(pend) > AV_TRAIL:
                        # prefer draining ACT-exp'd tiles; Schraudolph tiles
                        # (DVE, often late) defer to the unit tail so the PE
                        # never waits on them mid-unit (SCH_LAST).
                        idx = 0
                        if SCH_LAST:
                            for ei, e_ in enumerate(pend):
                                if not e_[2]:
                                    idx = ei
                                    break
                        e = pend.pop(idx)
                        emit_av(e[0], e[1], e[2], (0, 1), av01)
                    yield
                # The whole unit tail — qt0/qt1 drain, qt2/qt3 accumulation
                # and the feature-major transposes — is deferred into the NEXT
                # unit, staged after its first scores, so the PE never sits
                # behind exp(kp7) at a unit boundary.
                av23 = [psV.tile([128, D + 1], dt.float32, name=f"av{qt}", tag="av",
                                 padded_shape=[128, 512]) for qt in (2, 3)]

                def tail_av01():
                    for e in pend:
                        emit_av(e[0], e[1], e[2], (0, 1), av01)
                    normalize((0, 1), av01)

                def tail_av23():
                    for e in all_es:
                        emit_av(e[0], e[1], e[2], (2, 3), av23)
                    normalize((2, 3), av23)

                def tail_tp():
                    oTps = psA.tile([128, 4, 128], dt.bfloat16, name="oTps", tag="qkv")
                    for qt in range(4):
                        nc.tensor.transpose(oTps[off : off + 64, qt, :], o_sb[:, qt, :], identb[:])
                    nc.vector.tensor_copy(
                        oT[p][g][:, :].rearrange("p (qt c) -> p qt c", qt=4)[off : off + 64],
                        oTps[off : off + 64])
                tail_q.append((tail_av01, tail_av23, tail_tp))

            tail_q = []

            def drain_tails():
                while tail_q:
                    for f in tail_q.pop(0):
                        f()

            def drive_unit(st, g, hh, box, filler=None):
                # filler(kp) runs at every kpair boundary so prep/proj PE work
                # spreads inside the unit instead of bunching at its end; the
                # previous unit's deferred qt2/qt3 accumulation runs after this
                # unit's kp1 scores, its transposes one kpair later.
                stages = []
                for kp, _ in enumerate(att_unit_gen(st, g, hh, box, tail_q)):
                    if kp == 1:
                        while len(tail_q) > 1:
                            for f in tail_q.pop(0):
                                f()
                        if tail_q:
                            stages = list(tail_q.pop(0))
                    elif kp in (2, 3) and stages:
                        stages.pop(0)()
                    if kp == 1 and stages:
                        stages.pop(0)()
                    if filler is not None:
                        filler(kp)
                for f in stages:
                    f()

            def proj_steps(i):
                hold = {}

                def s1():
                    p512 = psA.tile([128, 512], dt.float32, tag="qkv")
                    for pp_ in range(3):
                        sl = oT[pp_][i // NG][:, (i % NG) * 128 : (i % NG + 1) * 128]
                        nc.tensor.matmul(p512[:], sl, prW[pp_][:, 0:512],
                                         start=(pp_ == 0), stop=(pp_ == 2))
                    os_ = outp.tile([128, C], dt.float32, tag="os")
                    nc.vector.tensor_copy(os_[:, 0:512], p512[:])
                    hold["os"] = os_

                def s2():
                    p256 = psA.tile([128, 256], dt.float32, tag="qkv")
                    for pp_ in range(3):
                        sl = oT[pp_][i // NG][:, (i % NG) * 128 : (i % NG + 1) * 128]
                        nc.tensor.matmul(p256[:], sl, prW[pp_][:, 512:768],
                                         start=(pp_ == 0), stop=(pp_ == 2))
                    os_ = hold["os"]
                    nc.vector.tensor_copy(os_[:, 512:768], p256[:])
                    nc.sync.dma_start(out[i * 128 : (i + 1) * 128, :], os_[:])

                return [s1, s2]

            def emit_proj_tile(i):
                for s in proj_steps(i):
                    s()

            # pair-0 prep up front (V matmuls included); the first attention
            # unit's kpairs are interleaved as soon as their kT/vA quads are
            # flushed, so the ACT exp stream starts early.
            cur = new_pair_state(0)
            box0 = {}
            gen0 = att_unit_gen(cur, 0, 0, box0, tail_q)
            gate = [max(3, 2 * j + 1) + FLUSH_LAG for j in range(8)]
            gate = [g if g <= NT - 1 else 99 for g in gate]
            kp_done = 0
            for i in range(NT):
                emit_prep_tile(cur)
                while kp_done < 8 and i >= gate[kp_done]:
                    next(gen0)
                    kp_done += 1
            finish_prep(cur, [])
            for _ in gen0:
                pass

            proj_queue = list(range(NT))
            for p in range(3):
                nxt = new_pair_state(p + 1) if p < 2 else None
                step_q = []

                def filler(g_cur, _nxt=nxt, _sq=step_q):
                    def f(kp):
                        if kp not in (FILL_KP if _nxt is not None else PROJ_KP):
                            return
                        if _nxt is not None:
                            if not _sq and _nxt["next"] < NT:
                                _sq.extend(prep_steps(_nxt))
                            if _sq:
                                _sq.pop(0)()
                        else:
                            if not _sq and proj_queue and proj_queue[0] < g_cur * NG:
                                _sq.extend(proj_steps(proj_queue.pop(0)))
                            if _sq:
                                _sq.pop(0)()
                    return f

                for g in range(NG):
                    box = box0 if (p == 0 and g == 0) else {}
                    for hh in range(2):
                        if p == 0 and g == 0 and hh == 0:
                            continue
                        drive_unit(cur, g, hh, box, filler(g))
                if nxt is not None:
                    finish_prep(nxt, step_q)
                    cur = nxt
                else:
                    while step_q:
                        step_q.pop(0)()
            drain_tails()
            last_steps = [proj_steps(i) for i in proj_queue]
            for s_idx in range(2):
                for ss in last_steps:
                    ss[s_idx]()
            xw_cm.__exit__(None, None, None)

    nc.compile()
    return nc


_NC = None


def _get_nc():
    global _NC
    if _NC is None:
        _NC = build_program()
    return _NC


def qkv_np_dt():
    import ml_dtypes
    return ml_dtypes.float8_e4m3 if QKV_FP8 else ml_dtypes.bfloat16


def _prep_inputs(x, cos, sin, qkv_w, q_norm_w, k_norm_w, proj_w):
    import ml_dtypes
    bf16 = ml_dtypes.bfloat16
    cos2 = np.asarray(cos, np.float32).reshape(N, D // 2)
    sin2 = np.asarray(sin, np.float32).reshape(N, D // 2)
    cos_full = np.concatenate([cos2, cos2], axis=1)          # [N, 64]
    sin_signed = np.concatenate([-sin2, sin2], axis=1)       # [N, 64]

    def tables(w):
        w = np.asarray(w, np.float32)
        wswap = np.concatenate([w[D // 2 :], w[: D // 2]])
        cw = (8.0 * cos_full * w[None, :]).astype(np.float32)
        sw = (8.0 * sin_signed * wswap[None, :]).astype(np.float32)
        return np.ascontiguousarray(cw), np.ascontiguousarray(sw)

    cwq_, swq_ = tables(q_norm_w)
    cwk_, swk_ = tables(k_norm_w)
    cqk_ = np.ascontiguousarray(np.stack([cwq_, cwk_], axis=1).reshape(N, 2 * D)).astype(bf16)
    sqk_ = np.ascontiguousarray(np.stack([swq_, swk_], axis=1).reshape(N, 2 * D)).astype(bf16)

    in_maps = []
    for c in range(8):
        b, hg = c // 2, c % 2
        h0 = HPC * hg
        rows = np.r_[h0 * D : (h0 + HPC) * D]
        wq = qkv_w[rows]          # [384, C]
        wk = qkv_w[C + rows]
        wv = qkv_w[2 * C + rows]
        # pack as [q0|k0 (256), v (384), q1|k1, q2|k2]
        parts = [wq[0:128], wk[0:128], wv]
        for p in range(1, 3):
            parts.append(wq[p * 128 : (p + 1) * 128])
            parts.append(wk[p * 128 : (p + 1) * 128])
        qdt = qkv_np_dt()
        wqkvT_ = np.ascontiguousarray(np.concatenate(parts, 0).T).astype(qdt)
        projT_ = np.ascontiguousarray(proj_w[:, rows].T).astype(bf16)
        xT_ = np.ascontiguousarray(x[b].T).astype(qdt)
        m = {
            "xT": xT_, "wqkvT": wqkvT_, "projT": projT_,
            "cqk": cqk_, "sqk": sqk_,
        }
        if V_FP8:
            e4 = ml_dtypes.float8_e4m3
            m["xT8"] = np.ascontiguousarray(x[b].T).astype(e4)
            m["wv8"] = np.ascontiguousarray(wv.T).astype(e4)
        in_maps.append(m)
    return in_maps


def kernel(x, cos, sin, qkv_w, q_norm_w, k_norm_w, proj_w, proj_b, _want_trace=False):
    x = np.asarray(x, np.float32)
    qkv_w = np.asarray(qkv_w, np.float32)
    proj_w = np.asarray(proj_w, np.float32)
    proj_b = np.asarray(proj_b, np.float32)
    in_maps = _prep_inputs(x, cos, sin, qkv_w, q_norm_w, k_norm_w, proj_w)
    nc = _get_nc()
    res = run_bass_kernel_spmd(nc, in_maps, core_ids=list(range(8)), trace=_want_trace)
    out = np.empty((B, N, C), np.float32)
    for b in range(B):
        out[b] = res.results[2 * b]["out"] + res.results[2 * b + 1]["out"] + proj_b[None, :]
    if _want_trace:
        return out, res
    return out



# revision 31
# speedup vs baseline: 1.0277x; 1.0036x over previous
"""Trainium2 Bass kernel for nn_Attention (B=4, N=2048, C=768, H=12, D=64).

Sharding: core c -> batch b=c//2, head-group hg=c%2 (6 heads each).
qkv_w column-parallel, proj_w row-parallel (host sums the 2 partials per b).

v4 structure (vs the f32r baseline):
  - QKV matmuls stay f32r (full rate, moving dims >= 256).
  - q/k tiles, rope tables, exp'd scores (es), V, attention output and the
    projection all run in bf16: transposes cost 1.0 cycles/row and the
    psum<->sbuf copies hit the DVE 2x half-word mode.
  - AV is FLIPPED: es [k,q] is the stationary operand, [v|1] the moving one,
    so each 128x128 score block costs 65 PE rows instead of 512. The ones
    column gives the softmax denominator in psum column 64. Output is
    token-major [q, d]; a bf16 PE transpose + copy rebuilds feature-major oT
    for the projection.
  - A matmul `start` zeroes its whole 2KB psum bank, so the 4 q-subtiles of a
    unit accumulate as qt0/qt1 in two ping-pong banks streaming with the
    exps, then qt2/qt3 re-use those banks; the qt0/1 drain, qt2/3
    accumulation and the transposes are deferred into the NEXT unit (staged
    after its first scores) so the PE never idles behind exp(kp7).
  - exp runs on ACT except 2 of 8 kpairs per unit, which run on DVE as ONE
    tensor_scalar: i16 = trunc(s*c0 + c1) bitcast to bf16 is a Schraudolph
    exp (max rel err ~3% on those tiles; end-to-end ~9e-3 vs 2e-2 budget).
  - GPSIMD cannot touch PSUM: Pool only carries sbuf-side rope math
    (sq/t_/qn); all psum exits live on DVE/ACT.
  - rsqrt via bit-trick seed + 1 Newton step (0.2% worst-case, q-side
    cancels in softmax).
  - No max-subtraction needed: RMSNorm bounds the logits (|z| <= ~16).
"""
import sys

sys.path.insert(0, "/opt/trn_rl_repo")

import numpy as np
import concourse.bass as bass
import concourse.mybir as mybir
import concourse.tile as tile
from concourse import bacc
from concourse.bass_utils import run_bass_kernel_spmd
from concourse.masks import make_identity

dt = mybir.dt
AF = mybir.ActivationFunctionType
ALU = mybir.AluOpType
AX = mybir.AxisListType

B, N, C = 4, 2048, 768
H, D = 12, 64
HPC = 6            # heads per core
EPS = 1e-6
NT = N // 128      # 16 token tiles
NCHUNK = C // 128  # 6 contraction chunks
SCALE = D ** -0.5  # 0.125
NG = 4             # qi groups
G = N // NG        # 512 per group
# bf16 Schraudolph exp: i16 = trunc(s*SCH_C0 + SCH_C1); bitcast bf16 ~ exp(s/8)
SCH_C0 = 184.664965 * SCALE
SCH_C1 = 16250.5
import os
# kpair indices whose exp runs on DVE via Schraudolph ("" = none)
SCH_KP = tuple(int(x) for x in os.environ.get("SCH_KP", "5,7").split(",") if x != "")
FILL_KP = tuple(int(x) for x in os.environ.get("FILL_KP", "0,2,5,6").split(","))
AV_TRAIL = int(os.environ.get("AV_TRAIL", "5"))
PROJ_KP = tuple(int(x) for x in os.environ.get("PROJ_KP", "4,5,6,7").split(","))
ONE_NR = int(os.environ.get("ONE_NR", "1"))
M2_POOL = int(os.environ.get("M2_POOL", "0"))
SCH_DEFER = int(os.environ.get("SCH_DEFER", "0"))
FLUSH_LAG = int(os.environ.get("FLUSH_LAG", "10"))
TAIL_KP = int(os.environ.get("TAIL_KP", "0"))
RAMP_N = int(os.environ.get("RAMP_N", "16"))
ES_BUFS = int(os.environ.get("ES_BUFS", "14"))
# v5 knobs
QKV_FP8 = int(os.environ.get("QKV_FP8", "0"))   # x/w in fp8e4 + DoubleRow matmuls
V_FP8 = int(os.environ.get("V_FP8", "0"))       # fp8-DR for the V matmul only
# NOTE: walrus rejects TensorScalarPtr on Pool ("Instruction engine check
# failed (Pool)"), so the rsqrt chain must stay on DVE.
RSQRT_POOL = int(os.environ.get("RSQRT_POOL", "0"))
PM_DR = mybir.MatmulPerfMode.DoubleRow
NCH = 3 if QKV_FP8 else NCHUNK  # contraction chunks for qkv matmuls
# Schraudolph kpair patterns, cycled per attention unit: "4,7|3,4,7" alternates.
SCH_PAT = [tuple(int(x) for x in grp.split(",") if x != "")
           for grp in os.environ.get("SCH_PAT", "").split("|")] \
    if os.environ.get("SCH_PAT") else [SCH_KP]
QN_BUFS = int(os.environ.get("QN_BUFS", "20"))
RS_TT = int(os.environ.get("RS_TT", "0"))   # rsqrt chain as Pool TTs w/ const tiles
CP_DVE = int(os.environ.get("CP_DVE", "0"))  # qk_sb/vA psum->sbuf copies on DVE
SCH_LAST = int(os.environ.get("SCH_LAST", "0"))  # defer sch-kpair AV to unit tail
CH2 = int(os.environ.get("CH2", "0"))        # batch rsqrt chain over tile pairs


def _bc(ap, idx, count):
    """Insert a broadcast (step 0) free dim at position idx of an AP."""
    a = list(ap.ap)
    a.insert(idx, [0, count])
    return bass.AP(tensor=ap.tensor, offset=ap.offset, ap=a)


def build_program():
    nc = bacc.Bacc(None, target_bir_lowering=False)

    qkv_dt = dt.float8e4 if QKV_FP8 else dt.bfloat16
    xT = nc.dram_tensor("xT", [C, N], qkv_dt, kind="ExternalInput")
    # host layout: [q0|k0 (256) | v (384) | q1|k1 | q2|k2]
    wqkvT = nc.dram_tensor("wqkvT", [C, 3 * HPC * D], qkv_dt, kind="ExternalInput")
    projT = nc.dram_tensor("projT", [HPC * D, C], dt.bfloat16, kind="ExternalInput")
    cqk = nc.dram_tensor("cqk", [N, 2 * D], dt.bfloat16, kind="ExternalInput")
    sqk = nc.dram_tensor("sqk", [N, 2 * D], dt.bfloat16, kind="ExternalInput")
    out = nc.dram_tensor("out", [N, C], dt.float32, kind="ExternalOutput")

    with tile.TileContext(nc) as tc:
        with (
            tc.tile_pool(name="persist", bufs=1) as persist,
            tc.tile_pool(name="qkrot", bufs=2) as qkrot,     # qT/kT rotate across pairs
            tc.tile_pool(name="work", bufs=3) as work,
            tc.tile_pool(name="qkblk", bufs=2) as qkblk,
            tc.tile_pool(name="tiny", bufs=2) as tiny,
            tc.tile_pool(name="den", bufs=2) as den,
            tc.tile_pool(name="p2e", bufs=ES_BUFS) as p2e,
            tc.tile_pool(name="outp", bufs=4) as outp,
            tc.tile_pool(name="psA", bufs=2, space="PSUM") as psA,   # qkv/tp/proj
            tc.tile_pool(name="psS", bufs=2, space="PSUM") as psS,   # scores
            tc.tile_pool(name="psV", bufs=2, space="PSUM") as psV,   # av + oT transposes
        ):
            # ---------------- persistent tiles --------------------------------
            oT = [[persist.tile([128, G], dt.bfloat16, name=f"oT{p}_{g}", tag=f"oT{p}_{g}")
                   for g in range(NG)] for p in range(3)]
            vA = [persist.tile([128, 4, HPC, D + 1], dt.bfloat16, name=f"vA{kg}", tag=f"vA{kg}")
                  for kg in range(NG)]
            identb = persist.tile([128, 128], dt.bfloat16, tag="identb")
            make_identity(nc, identb[:])
            ones1 = persist.tile([128, 1], dt.float32, tag="ones1")
            nc.vector.memset(ones1[:], 1.0)
            for kg in range(NG):
                nc.vector.tensor_copy(vA[kg][:, :, :, D : D + 1], _bc(_bc(ones1[:], 1, 4), 2, HPC))
            # broadcast constants for the Pool-TT rsqrt chain (RS_TT)
            rsc = {}
            if RS_TT:
                for nm, val, cdt in (("one_i", 1, dt.int32), ("magic", 0x5F3759DF, dt.int32),
                                     ("halfc", 0.5, dt.float32), ("c15", 1.5, dt.float32)):
                    t_c = persist.tile([128, 1], cdt, tag=f"rsc_{nm}")
                    nc.vector.memset(t_c[:], val)
                    rsc[nm] = t_c

            # weights / x^T / tables.  DMA order tuned so the first prep tiles
            # wait for the minimum byte set.
            xw_cm = tc.tile_pool(name="xw", bufs=1)
            xw = xw_cm.__enter__()
            CPC = C // NCH  # contraction rows per chunk (256 fp8-DR / 128 bf16)
            wrA = []
            wrB = []

            def _wsl(dram, j, lo, hi):
                sl = dram[j * CPC : (j + 1) * CPC, lo:hi]
                if QKV_FP8:
                    sl = sl.rearrange("(i p) f -> p i f", i=2)
                return sl

            xshp = [128, 2, G] if QKV_FP8 else [128, G]
            xr = [[xw.tile(list(xshp), qkv_dt, name=f"xr{j}_{tg}", tag=f"xr{j}_{tg}")
                   for tg in range(NG)] for j in range(NCH)]
            # interleave weight/x DMAs so chunk j's operands land together and
            # the first prep matmuls can start as early as possible
            for j in range(NCH):
                shp = [128, 2, 640] if QKV_FP8 else [128, 640]
                wa = xw.tile(shp, qkv_dt, name=f"wrA{j}", tag=f"wrA{j}")
                nc.sync.dma_start(wa[:], _wsl(wqkvT, j, 0, 640))
                wrA.append(wa)
                nc.sync.dma_start(xr[j][0][:], _wsl(xT, j, 0, G))
            tabs = {}
            for name, dram in (("cqk", cqk), ("sqk", sqk)):
                t = persist.tile([128, NT, 2, D], dt.bfloat16, name=name, tag=name)
                nc.sync.dma_start(t[:], dram.rearrange("(t p) (qk d) -> p t qk d", p=128, qk=2))
                tabs[name] = t
            for tg in range(1, NG):
                for j in range(NCH):
                    nc.sync.dma_start(xr[j][tg][:], _wsl(xT, j, tg * G, (tg + 1) * G))
            for j in range(NCH):
                shp = [128, 2, 512] if QKV_FP8 else [128, 512]
                wb = xw.tile(shp, qkv_dt, name=f"wrB{j}", tag=f"wrB{j}")
                nc.sync.dma_start(wb[:], _wsl(wqkvT, j, 640, 1152))
                wrB.append(wb)
            # V-only fp8: dedicated fp8 copies of x and the v weight columns,
            # used only by the V DoubleRow matmuls (q/k stay bf16).
            xr8, wrV = [], []
            if V_FP8:
                xT8 = nc.dram_tensor("xT8", [C, N], dt.float8e4, kind="ExternalInput")
                wv8 = nc.dram_tensor("wv8", [C, HPC * D], dt.float8e4, kind="ExternalInput")
                for j in range(3):
                    wv_ = xw.tile([128, 2, HPC * D], dt.float8e4, name=f"wrV{j}", tag=f"wrV{j}")
                    nc.sync.dma_start(wv_[:], wv8[j * 256 : (j + 1) * 256, :].rearrange("(i p) f -> p i f", i=2))
                    wrV.append(wv_)
                xr8 = [[xw.tile([128, 2, G], dt.float8e4, name=f"xr8_{j}_{tg}", tag=f"xr8_{j}_{tg}")
                        for tg in range(NG)] for j in range(3)]
                for tg in range(NG):
                    for j in range(3):
                        nc.sync.dma_start(xr8[j][tg][:], xT8[j * 256 : (j + 1) * 256, tg * G : (tg + 1) * G].rearrange("(i p) t -> p i t", i=2))
            prW = []
            for p in range(3):
                wp = persist.tile([128, C], dt.bfloat16, name=f"prW{p}", tag=f"prW{p}")
                nc.sync.dma_start(wp[:], projT[p * 128 : (p + 1) * 128, :])
                prW.append(wp)

            # ------- interleaved emission: prep / attention / projection ------
            # Engines execute their instruction streams in order, so emission
            # order IS the schedule.

            def new_pair_state(p):
                # qkT[g]: columns 0:G hold q^T for qi-group g, G:2G hold k^T
                # for ki-group g.  bf16.
                return {
                    "p": p,
                    "qkT": [qkrot.tile([128, 2 * G], dt.bfloat16, name=f"qkT{p}_{g}", tag=f"qkT{g}") for g in range(NG)],
                    "pend": [],
                    "next": 0,
                }

            def flush_one(st):
                i, qn = st["pend"].pop(0)
                # both bf16 transposes land in one psum tile -> single 2x copy
                tp = psA.tile([128, 256], dt.bfloat16, tag="qkv")
                nc.tensor.transpose(tp[:, 0:128], qn[:, 0:128], identb[:])
                nc.tensor.transpose(tp[:, 128:256], qn[:, 128:256], identb[:])
                dst = st["qkT"][i // NG][:, :].rearrange("p (qk c) -> p qk c", qk=2)[
                    :, :, (i % NG) * 128 : (i % NG + 1) * 128]
                src_v = tp[:, :].rearrange("p (qk c) -> p qk c", qk=2)
                nc.vector.tensor_copy(dst, src_v)

            _pm = PM_DR if QKV_FP8 else None
            _S1Q = 1 if QKV_FP8 else 3  # qk chunks emitted in s1

            def _xsl(j, i):
                x_t = xr[j][i // NG]
                lo, hi = (i % NG) * 128, (i % NG + 1) * 128
                return x_t[:, :, lo:hi] if QKV_FP8 else x_t[:, lo:hi]

            def _wsel(p, j, lo, hi):
                # p==0: q0|k0 in wrA[:, 0:256); p>0: qp|kp in wrB[:, (p-1)*256:p*256)
                wt = wrA[j] if p == 0 else wrB[j]
                if p != 0:
                    lo, hi = (p - 1) * 256, p * 256
                return wt[:, :, lo:hi] if QKV_FP8 else wt[:, lo:hi]

            def prep_steps(st):
                # Split one prep tile into two PE bursts so the filler can
                # spread them between attention kpairs.
                p = st["p"]
                i = st["next"]
                st["next"] += 1
                hold = {}

                def s1():
                    if p == 0:
                        vp = psA.tile([128, HPC * D], dt.float32, tag="qkv")
                        if V_FP8:
                            for j in range(3):
                                nc.tensor.matmul(vp[:], xr8[j][i // NG][:, :, (i % NG) * 128 : (i % NG + 1) * 128],
                                                 wrV[j][:], start=(j == 0), stop=(j == 2),
                                                 perf_mode=PM_DR)
                        else:
                            for j in range(NCH):
                                wsl = wrA[j][:, :, 256:640] if QKV_FP8 else wrA[j][:, 256:640]
                                nc.tensor.matmul(vp[:], _xsl(j, i), wsl,
                                                 start=(j == 0), stop=(j == NCH - 1),
                                                 perf_mode=_pm)
                        if CP_DVE:
                            nc.vector.tensor_copy(vA[i // NG][:, i % NG, :, 0:D], vp[:].rearrange("p (h d) -> p h d", h=HPC))
                        else:
                            nc.scalar.copy(vA[i // NG][:, i % NG, :, 0:D], vp[:].rearrange("p (h d) -> p h d", h=HPC))
                    qkp = psA.tile([128, 256], dt.float32, tag="qkv")
                    for j in range(_S1Q):
                        nc.tensor.matmul(qkp[:], _xsl(j, i), _wsel(p, j, 0, 256),
                                         start=(j == 0), stop=False, perf_mode=_pm)
                    hold["qkp"] = qkp

                def s2():
                    _finish_prep_tile(st, i, hold["qkp"])

                return [s1, s2]

            def _finish_prep_tile(st, i, qkp):
                p = st["p"]
                for j in range(_S1Q, NCH):
                    nc.tensor.matmul(qkp[:], _xsl(j, i), _wsel(p, j, 0, 256),
                                     start=False, stop=(j == NCH - 1), perf_mode=_pm)
                if len(st["pend"]) >= FLUSH_LAG:
                    flush_one(st)
                qk_sb = qkblk.tile([128, 256], dt.bfloat16, tag="qk_sb")
                if CP_DVE:
                    nc.vector.tensor_copy(qk_sb[:], qkp[:])
                else:
                    nc.scalar.copy(qk_sb[:], qkp[:])
                qk4 = qk_sb[:].rearrange("p (h d) -> p h d", h=4)
                # sum of squares per (token, slot).  During the pair-0 ramp
                # (no exps yet) the otherwise-idle ACT computes the squares
                # and DVE the rope add, halving Pool's prep throughput limit.
                ramp = (p == 0 and i < RAMP_N)
                sq = work.tile([128, 4, D], dt.bfloat16, tag="sq")
                if ramp:
                    nc.scalar.square(sq[:], qk4)
                else:
                    nc.gpsimd.tensor_tensor(sq[:], qk4, qk4, op=ALU.mult)
                if CH2:
                    if i % 2 == 0:
                        ss2 = tiny.tile([128, 2, 4], dt.float32, tag="ss16")
                        st["ss2"] = ss2
                        st["half"] = (i, qk4, ramp)
                        nc.vector.tensor_reduce(ss2[:, 0], sq[:], axis=AX.X, op=ALU.add)
                        return
                    ss2 = st["ss2"]
                    nc.vector.tensor_reduce(ss2[:, 1], sq[:], axis=AX.X, op=ALU.add)
                    nf8 = tiny.tile([128, 2, 4], dt.float32, tag="nf16")
                    _chain(ss2[:, :, :], nf8[:, :, :], [2, 4], ramp)
                    i0, qk40, ramp0 = st["half"]
                    _rope(st, i0, qk40, nf8[:, 0], ramp0)
                    _rope(st, i, qk4, nf8[:, 1], ramp)
                else:
                    ss = tiny.tile([128, 4], dt.float32, tag="ss16")
                    nc.vector.tensor_reduce(ss[:], sq[:], axis=AX.X, op=ALU.add)
                    nf16 = tiny.tile([128, 4], dt.float32, tag="nf16")
                    _chain(ss[:], nf16[:], [4], ramp)
                    _rope(st, i, qk4, nf16[:], ramp)

            def _chain(ss, nf16, tail, ramp):
                # rsqrt (bit-trick + Newton) on DVE: nf = 1/sqrt(ss+D*EPS)
                shape = [128] + list(tail)
                rs = nc.vector
                ssh = tiny.tile(shape, dt.float32, tag="ssh")
                rs.tensor_scalar(ssh[:], ss, 0.5, 0.5 * D * EPS,
                                 op0=ALU.mult, op1=ALU.add)
                y0i = tiny.tile(shape, dt.int32, tag="y0i")
                rs.tensor_scalar(y0i[:], ss.bitcast(dt.int32), 1, 0,
                                 op0=ALU.logical_shift_right, op1=ALU.bitwise_or)
                rs.tensor_scalar(y0i[:], y0i[:], -1, 0x5F3759DF,
                                 op0=ALU.mult, op1=ALU.add)
                yw = tiny.tile(shape, dt.float32, tag="yw")
                y = y0i[:].bitcast(dt.float32)
                for dst_ in (nf16,) if ONE_NR else (tiny.tile(shape, dt.float32, tag="y1")[:], nf16):
                    rs.tensor_tensor(yw[:], y, y, op=ALU.mult)
                    rs.tensor_tensor(yw[:], yw[:], ssh[:], op=ALU.mult)
                    rs.tensor_scalar(yw[:], yw[:], -1.0, 1.5,
                                     op0=ALU.mult, op1=ALU.add)
                    rs.tensor_tensor(dst_, y, yw[:], op=ALU.mult)
                    y = dst_

            def _rope(st, i, qk4, nf16, ramp):
                nfb = _bc(nf16, 2, D)
                t_ = work.tile([128, 4, D], dt.bfloat16, tag="t_")
                nc.gpsimd.tensor_tensor(t_[:], qk4, nfb, op=ALU.mult)
                # rope tables: [128, NT, 2(qk), D] bf16 with heads broadcast
                cwb = _bc(tabs["cqk"][:, i, :, :], 2, 2)
                swb = _bc(tabs["sqk"][:, i, :, :], 2, 2)
                t4 = t_[:].rearrange("p (qk h) d -> p qk h d", qk=2)
                h_ = D // 2
                m1 = work.tile([128, 2, 2, D], dt.bfloat16, tag="m1")
                nc.vector.tensor_tensor(m1[:], t4, cwb, op=ALU.mult)
                m2 = work.tile([128, 2, 2, D], dt.bfloat16, tag="m2")
                m2eng = nc.gpsimd if M2_POOL else nc.vector
                m2eng.tensor_tensor(m2[:, :, :, 0:h_], t4[:, :, :, h_:D], swb[:, :, :, 0:h_], op=ALU.mult)
                m2eng.tensor_tensor(m2[:, :, :, h_:D], t4[:, :, :, 0:h_], swb[:, :, :, h_:D], op=ALU.mult)
                qn = work.tile([128, 256], dt.bfloat16, tag="qn", bufs=QN_BUFS)
                qn_eng = nc.vector if ramp else nc.gpsimd
                qn_eng.tensor_tensor(qn[:].rearrange("p (qk h d) -> p qk h d", qk=2, h=2), m1[:], m2[:], op=ALU.add)
                st["pend"].append((i, qn))

            def emit_prep_tile(st):
                for s in prep_steps(st):
                    s()

            def finish_prep(st, step_q):
                while step_q:
                    step_q.pop(0)()
                while st["next"] < NT:
                    emit_prep_tile(st)
                while st["pend"]:
                    flush_one(st)

            unit_no = [0]

            def att_unit_gen(st, g, hh, oTps_box, tail_q):
                p = st["p"]
                h = 2 * p + hh
                off = 64 * hh
                sch_set = SCH_PAT[unit_no[0] % len(SCH_PAT)]
                unit_no[0] += 1
                # Two accumulation banks ping-pong across the 4 q-tiles: a
                # matmul start zeroes its whole 2KB psum bank, so concurrent
                # groups must sit in different banks.  qt0/qt1 stream with the
                # exps; qt2/qt3 re-use the banks at the unit end.
                av01 = [psV.tile([128, D + 1], dt.float32, name=f"av{qt}", tag="av",
                                 padded_shape=[128, 512]) for qt in range(2)]

                def emit_av(kpair, es, is_i16, qts, avt):
                    for half in range(2):
                        ki = kpair * 2 + half
                        for j, qt in enumerate(qts):
                            esb = es[:, half * 512 + qt * 128 : half * 512 + (qt + 1) * 128]
                            if is_i16:
                                esb = esb.bitcast(dt.bfloat16)
                            nc.tensor.matmul(
                                avt[j][:],
                                esb,
                                vA[ki // NG][:, ki % NG, h, :],
                                start=(ki == 0), stop=(ki == NT - 1),
                            )

                rd4 = den.tile([128, 4], dt.float32, tag="rd4")
                o_sb = den.tile([128, 4, D], dt.bfloat16, tag="o_sb")

                def normalize(qts, avt):
                    for j, qt in enumerate(qts):
                        nc.vector.reciprocal(rd4[:, qt : qt + 1], avt[j][:, D : D + 1])
                        nc.vector.tensor_scalar(o_sb[:, qt, :], avt[j][:, 0:D],
                                                rd4[:, qt : qt + 1], None, op0=ALU.mult)

                pend = []
                all_es = []
                for kpair in range(8):
                    sp = psS.tile([128, 1024], dt.float32, tag="sp")
                    for half in range(2):
                        ki = kpair * 2 + half
                        nc.tensor.matmul(
                            sp[:, half * 512 : (half + 1) * 512],
                            st["qkT"][ki // NG][off : off + 64, G + (ki % NG) * 128 : G + (ki % NG + 1) * 128],
                            st["qkT"][g][off : off + 64, 0:G],
                            start=True, stop=True,
                        )
                    # exp: mostly on ACT; SCH_KP kpairs on DVE via the bf16
                    # Schraudolph bit-trick (one tensor_scalar; GPSIMD cannot
                    # read PSUM so Pool is out).
                    if kpair in sch_set:
                        esi = p2e.tile([128, 1024], dt.int16, name="esi", tag="es")
                        nc.vector.tensor_scalar(esi[:], sp[:], SCH_C0, SCH_C1,
                                                op0=ALU.mult, op1=ALU.add)
                        entry = (kpair, esi, True)
                    else:
                        es = p2e.tile([128, 1024], dt.bfloat16, name="est", tag="es")
                        nc.scalar.activation(es[:], sp[:], AF.Exp, scale=SCALE)
                        entry = (kpair, es, False)
                    pend.append(entry)
                    all_es.append(entry)
                    if len(pend) > AV_TRAIL:
                        # prefer draining ACT-exp'd tiles; Schraudolph tiles
                        # (DVE, often late) defer to the unit tail so the PE
                        # never waits on them mid-unit (SCH_LAST).
                        idx = 0
                        if SCH_LAST:
                            for ei, e_ in enumerate(pend):
                                if not e_[2]:
                                    idx = ei
                                    break
                        e = pend.pop(idx)
                        emit_av(e[0], e[1], e[2], (0, 1), av01)
                    yield
                # The whole unit tail — qt0/qt1 drain, qt2/qt3 accumulation
                # and the feature-major transposes — is deferred into the NEXT
                # unit, staged after its first scores, so the PE never sits
                # behind exp(kp7) at a unit boundary.
                av23 = [psV.tile([128, D + 1], dt.float32, name=f"av{qt}", tag="av",
                                 padded_shape=[128, 512]) for qt in (2, 3)]

                def tail_av01():
                    for e in pend:
                        emit_av(e[0], e[1], e[2], (0, 1), av01)
                    normalize((0, 1), av01)

                def tail_av23():
                    for e in all_es:
                        emit_av(e[0], e[1], e[2], (2, 3), av23)
                    normalize((2, 3), av23)

                def tail_tp():
                    oTps = psA.tile([128, 4, 128], dt.bfloat16, name="oTps", tag="qkv")
                    for qt in range(4):
                        nc.tensor.transpose(oTps[off : off + 64, qt, :], o_sb[:, qt, :], identb[:])
                    nc.vector.tensor_copy(
                        oT[p][g][:, :].rearrange("p (qt c) -> p qt c", qt=4)[off : off + 64],
                        oTps[off : off + 64])
                tail_q.append((tail_av01, tail_av23, tail_tp))

            tail_q = []

            def drain_tails():
                while tail_q:
                    for f in tail_q.pop(0):
                        f()

            def drive_unit(st, g, hh, box, filler=None):
                # filler(kp) runs at every kpair boundary so prep/proj PE work
                # spreads inside the unit instead of bunching at its end; the
                # previous unit's deferred qt2/qt3 accumulation runs after this
                # unit's kp1 scores, its transposes one kpair later.
                stages = []
                for kp, _ in enumerate(att_unit_gen(st, g, hh, box, tail_q)):
                    if kp == 1:
                        while len(tail_q) > 1:
                            for f in tail_q.pop(0):
                                f()
                        if tail_q:
                            stages = list(tail_q.pop(0))
                    elif kp in (2, 3) and stages:
                        stages.pop(0)()
                    if kp == 1 and stages:
                        stages.pop(0)()
                    if filler is not None:
                        filler(kp)
                for f in stages:
                    f()

            def proj_steps(i):
                hold = {}

                def s1():
                    p512 = psA.tile([128, 512], dt.float32, tag="qkv")
                    for pp_ in range(3):
                        sl = oT[pp_][i // NG][:, (i % NG) * 128 : (i % NG + 1) * 128]
                        nc.tensor.matmul(p512[:], sl, prW[pp_][:, 0:512],
                                         start=(pp_ == 0), stop=(pp_ == 2))
                    os_ = outp.tile([128, C], dt.float32, tag="os")
                    nc.vector.tensor_copy(os_[:, 0:512], p512[:])
                    hold["os"] = os_

                def s2():
                    p256 = psA.tile([128, 256], dt.float32, tag="qkv")
                    for pp_ in range(3):
                        sl = oT[pp_][i // NG][:, (i % NG) * 128 : (i % NG + 1) * 128]
                        nc.tensor.matmul(p256[:], sl, prW[pp_][:, 512:768],
                                         start=(pp_ == 0), stop=(pp_ == 2))
                    os_ = hold["os"]
                    nc.vector.tensor_copy(os_[:, 512:768], p256[:])
                    nc.sync.dma_start(out[i * 128 : (i + 1) * 128, :], os_[:])

                return [s1, s2]

            def emit_proj_tile(i):
                for s in proj_steps(i):
                    s()

            # pair-0 prep up front (V matmuls included); the first attention
            # unit's kpairs are interleaved as soon as their kT/vA quads are
            # flushed, so the ACT exp stream starts early.
            cur = new_pair_state(0)
            box0 = {}
            gen0 = att_unit_gen(cur, 0, 0, box0, tail_q)
            gate = [max(3, 2 * j + 1) + FLUSH_LAG for j in range(8)]
            gate = [g if g <= NT - 1 else 99 for g in gate]
            kp_done = 0
            for i in range(NT):
                emit_prep_tile(cur)
                while kp_done < 8 and i >= gate[kp_done]:
                    next(gen0)
                    kp_done += 1
            finish_prep(cur, [])
            for _ in gen0:
                pass

            proj_queue = list(range(NT))
            for p in range(3):
                nxt = new_pair_state(p + 1) if p < 2 else None
                step_q = []

                def filler(g_cur, _nxt=nxt, _sq=step_q):
                    def f(kp):
                        if kp not in (FILL_KP if _nxt is not None else PROJ_KP):
                            return
                        if _nxt is not None:
                            if not _sq and _nxt["next"] < NT:
                                _sq.extend(prep_steps(_nxt))
                            if _sq:
                                _sq.pop(0)()
                        else:
                            if not _sq and proj_queue and proj_queue[0] < g_cur * NG:
                                _sq.extend(proj_steps(proj_queue.pop(0)))
                            if _sq:
                                _sq.pop(0)()
                    return f

                for g in range(NG):
                    box = box0 if (p == 0 and g == 0) else {}
                    for hh in range(2):
                        if p == 0 and g == 0 and hh == 0:
                            continue
                        drive_unit(cur, g, hh, box, filler(g))
                if nxt is not None:
                    finish_prep(nxt, step_q)
                    cur = nxt
                else:
                    while step_q:
                        step_q.pop(0)()
            drain_tails()
            last_steps = [proj_steps(i) for i in proj_queue]
            for s_idx in range(2):
                for ss in last_steps:
                    ss[s_idx]()
            xw_cm.__exit__(None, None, None)

    nc.compile()
    return nc


_NC = None


def _get_nc():
    global _NC
    if _NC is None:
        _NC = build_program()
    return _NC


def qkv_np_dt():
    import ml_dtypes
    return ml_dtypes.float8_e4m3 if QKV_FP8 else ml_dtypes.bfloat16


def _prep_inputs(x, cos, sin, qkv_w, q_norm_w, k_norm_w, proj_w):
    import ml_dtypes
    bf16 = ml_dtypes.bfloat16
    cos2 = np.asarray(cos, np.float32).reshape(N, D // 2)
    sin2 = np.asarray(sin, np.float32).reshape(N, D // 2)
    cos_full = np.concatenate([cos2, cos2], axis=1)          # [N, 64]
    sin_signed = np.concatenate([-sin2, sin2], axis=1)       # [N, 64]

    def tables(w):
        w = np.asarray(w, np.float32)
        wswap = np.concatenate([w[D // 2 :], w[: D // 2]])
        cw = (8.0 * cos_full * w[None, :]).astype(np.float32)
        sw = (8.0 * sin_signed * wswap[None, :]).astype(np.float32)
        return np.ascontiguousarray(cw), np.ascontiguousarray(sw)

    cwq_, swq_ = tables(q_norm_w)
    cwk_, swk_ = tables(k_norm_w)
    cqk_ = np.ascontiguousarray(np.stack([cwq_, cwk_], axis=1).reshape(N, 2 * D)).astype(bf16)
    sqk_ = np.ascontiguousarray(np.stack([swq_, swk_], axis=1).reshape(N, 2 * D)).astype(bf16)

    in_maps = []
    for c in range(8):
        b, hg = c // 2, c % 2
        h0 = HPC * hg
        rows = np.r_[h0 * D : (h0 + HPC) * D]
        wq = qkv_w[rows]          # [384, C]
        wk = qkv_w[C + rows]
        wv = qkv_w[2 * C + rows]
        # pack as [q0|k0 (256), v (384), q1|k1, q2|k2]
        parts = [wq[0:128], wk[0:128], wv]
        for p in range(1, 3):
            parts.append(wq[p * 128 : (p + 1) * 128])
            parts.append(wk[p * 128 : (p + 1) * 128])
        qdt = qkv_np_dt()
        wqkvT_ = np.ascontiguousarray(np.concatenate(parts, 0).T).astype(qdt)
        projT_ = np.ascontiguousarray(proj_w[:, rows].T).astype(bf16)
        xT_ = np.ascontiguousarray(x[b].T).astype(qdt)
        m = {
            "xT": xT_, "wqkvT": wqkvT_, "projT": projT_,
            "cqk": cqk_, "sqk": sqk_,
        }
        if V_FP8:
            e4 = ml_dtypes.float8_e4m3
            m["xT8"] = np.ascontiguousarray(x[b].T).astype(e4)
            m["wv8"] = np.ascontiguousarray(wv.T).astype(e4)
        in_maps.append(m)
    return in_maps


def kernel(x, cos, sin, qkv_w, q_norm_w, k_norm_w, proj_w, proj_b, _want_trace=False):
    x = np.asarray(x, np.float32)
    qkv_w = np.asarray(qkv_w, np.float32)
    proj_w = np.asarray(proj_w, np.float32)
    proj_b = np.asarray(proj_b, np.float32)
    in_maps = _prep_inputs(x, cos, sin, qkv_w, q_norm_w, k_norm_w, proj_w)
    nc = _get_nc()
    res = run_bass_kernel_spmd(nc, in_maps, core_ids=list(range(8)), trace=_want_trace)
    out = np.empty((B, N, C), np.float32)
    for b in range(B):
        out[b] = res.results[2 * b]["out"] + res.results[2 * b + 1]["out"] + proj_b[None, :]
    if _want_trace:
        return out, res
    return out

